# revision 1
# baseline (speedup 1.0000x reference)
"""Multi-head attention on 8 Trainium2 NeuronCores.

Problem: B=2, S=2048, D=1024, H=16 heads (head_dim 64), boolean mask,
per-head gate, QKV/out linear projections.

Sharding: core c handles batch b=c//4 and heads 4*(c%4)..4*(c%4)+3.
Each core computes its 4 heads' attention and the partial output
projection (contribution of its 256 concat columns through Wo); the host
sums the 4 partials per batch and adds the constant terms (bo, and the
bv/gate contribution which is constant because attention rows sum to 1).

Device-side layout choices (see comments inline):
  - scores are computed TRANSPOSED [sk, sq] so that softmax needs no
    free-dim reductions at all: exp is a pure elementwise ACT pass,
    the mask is a multiplicative bf16 tensor_mul, and the softmax
    denominator is obtained for free as a 65th "ones" column of the
    PV matmul's stationary operand.
  - all matmuls run in bf16 (1 cycle/row on the PE).
  - normalization divides the PV accumulator by the denominator row via
    reciprocal_approx_fast + gpsimd partition_broadcast + tensor_mul.
"""

import sys

if "/opt/trn_rl_repo" not in sys.path:
    sys.path.insert(0, "/opt/trn_rl_repo")

import numpy as np
import ml_dtypes

import concourse.bass as bass
import concourse.bacc as bacc
import concourse.mybir as mybir
import concourse.tile as tile
from concourse.bass_utils import run_bass_kernel_spmd

BF16 = mybir.dt.float16  # fp16: same speed as bf16, 3 more mantissa bits
F32 = mybir.dt.float32
NPBF16 = np.float16

P = 128
B, S, D = 2, 2048, 1024
HEADS, HD = 16, 64
NCORES = 8
NH = HEADS // (NCORES // B)  # heads per core = 4
COLS = NH * HD               # 256 concat columns per core
DK = D // P                  # 8 contraction chunks for the projections
SKT = S // P                 # 16 key chunks
SQB = 1024                   # query block width in the attention loop
NSQB = S // SQB

_CACHE = {}


def _build_program():
    nc = bacc.Bacc("TRN2", debug=False)

    xqT = nc.declare_dram_parameter("xqT", [D, S], BF16, isOutput=False)
    xkT = nc.declare_dram_parameter("xkT", [D, S], BF16, isOutput=False)
    xvT = nc.declare_dram_parameter("xvT", [D, S], BF16, isOutput=False)
    mT = nc.declare_dram_parameter("mT", [S, S], BF16, isOutput=False)
    wq = nc.declare_dram_parameter("wq", [D, COLS], BF16, isOutput=False)
    wk = nc.declare_dram_parameter("wk", [D, COLS], BF16, isOutput=False)
    wv = nc.declare_dram_parameter("wv", [D, COLS], BF16, isOutput=False)
    wo = nc.declare_dram_parameter("wo", [COLS, D], BF16, isOutput=False)
    bq = nc.declare_dram_parameter("bq", [COLS, 1], F32, isOutput=False)
    bk = nc.declare_dram_parameter("bk", [COLS, 1], F32, isOutput=False)
    od = nc.declare_dram_parameter("od", [D, S], F32, isOutput=True)

    xqT3 = xqT[:].rearrange("(n p) s -> n p s", p=P)
    xkT3 = xkT[:].rearrange("(n p) s -> n p s", p=P)
    xvT3 = xvT[:].rearrange("(n p) s -> n p s", p=P)
    mT3 = mT[:].rearrange("(n p) s -> n p s", p=P)
    wq3 = wq[:].rearrange("(n p) c -> n p c", p=P)
    wk3 = wk[:].rearrange("(n p) c -> n p c", p=P)
    wv3 = wv[:].rearrange("(n p) c -> n p c", p=P)
    wo3 = wo[:].rearrange("(n p) d -> n p d", p=P)
    bq3 = bq[:].rearrange("(n p) o -> n p o", p=P)
    bk3 = bk[:].rearrange("(n p) o -> n p o", p=P)
    od3 = od[:].rearrange("(n p) s -> n p s", p=P)

    with tile.TileContext(nc) as tc:
        with (
            tc.tile_pool(name="wpool", bufs=1) as wpool,
            tc.tile_pool(name="qkpool", bufs=1) as qkpool,
            tc.tile_pool(name="vpool", bufs=1) as vpool,
            tc.tile_pool(name="maskpool", bufs=1) as maskpool,
            tc.tile_pool(name="cpool", bufs=1) as cpool,
            tc.tile_pool(name="xpool", bufs=1) as xpool,
            tc.tile_pool(name="pmpool", bufs=1) as pmpool,
            tc.tile_pool(name="npool", bufs=1) as npool,
            tc.tile_pool(name="opool", bufs=1) as opool,
        ):
            # ---- resident weights / biases ----
            wq_sb, wk_sb, wv_sb = [], [], []
            for i in range(DK):
                t = wpool.tile([P, COLS], BF16, name=f"wq_sb{i}")
                nc.gpsimd.dma_start(out=t[:], in_=wq3[i])
                wq_sb.append(t)
            for i in range(DK):
                t = wpool.tile([P, COLS], BF16, name=f"wk_sb{i}")
                nc.gpsimd.dma_start(out=t[:], in_=wk3[i])
                wk_sb.append(t)
            for i in range(DK):
                t = wpool.tile([P, COLS], BF16, name=f"wv_sb{i}")
                nc.gpsimd.dma_start(out=t[:], in_=wv3[i])
                wv_sb.append(t)
            wo_sb = []
            for i in range(COLS // P):
                t = wpool.tile([P, D], BF16, name=f"wo_sb{i}")
                nc.gpsimd.dma_start(out=t[:], in_=wo3[i])
                wo_sb.append(t)
            b_sb = {}
            for nm, src in (("bq", bq3), ("bk", bk3)):
                for i in range(COLS // P):
                    t = wpool.tile([P, 1], F32, name=f"{nm}_sb{i}")
                    nc.gpsimd.dma_start(out=t[:], in_=src[i])
                    b_sb[(nm, i)] = t

            # concat^T (normalized attention outputs, head-major columns)
            concat_sb = [
                cpool.tile([P, S], BF16, name=f"concat_sb{i}")
                for i in range(COLS // P)
            ]

            # ---- Q/K projections: qhT[c, s] = (q @ Wq + bq)^T ----
            # lhsT = Wq chunk [128d, 128c] (stationary), rhs = xT chunk
            # [128d, 512s] -> psum [128c, 512s]; accumulate over 8 d-chunks.
            qhT_sb = {}
            with tc.tile_pool(name="ps_proj", bufs=1, space="PSUM") as psp:
                for tname, x3, w_sb, dest in (
                    ("q", xqT3, wq_sb, "qhT"),
                    ("k", xkT3, wk_sb, "khT"),
                ):
                    ps = [
                        psp.tile([P, S], F32, name=f"psp{c}", tag=f"psp{c}")
                        for c in range(COLS // P)
                    ]
                    for dk in range(DK):
                        xt = xpool.tile([P, S], BF16, name="xt", tag="xt", bufs=8)
                        xeng = nc.scalar if tname == "k" else nc.sync
                        xeng.dma_start(out=xt[:], in_=x3[dk])
                        for c in range(COLS // P):
                            for sb in range(S // 512):
                                nc.tensor.matmul(
                                    ps[c][:, sb * 512 : (sb + 1) * 512],
                                    lhsT=w_sb[dk][:, c * P : (c + 1) * P],
                                    rhs=xt[:, sb * 512 : (sb + 1) * 512],
                                    start=(dk == 0),
                                    stop=(dk == DK - 1),
                                )
                    for c in range(COLS // P):
                        t = qkpool.tile([P, S], BF16, name=f"{tname}hT{c}")
                        nc.vector.tensor_scalar_add(
                            t[:], ps[c][:], b_sb[(f"b{tname}", c)][:]
                        )
                        qhT_sb[(tname, c)] = t

            # Preload the ACT exp table set (~2.7us) while ScalarE is
            # otherwise idle, so the first attention exp doesn't pay it.
            warm = npool.tile([P, 1], F32, name="warm", tag="warm", bufs=1)
            nc.scalar.activation(
                warm[:], b_sb[("bq", 0)][:], mybir.ActivationFunctionType.Exp
            )

            # mask tiles, resident for the whole attention phase
            # (emitted after Q/K so the scalar queue serves xk first)
            m_sb = []
            for i in range(SKT):
                t = maskpool.tile([P, S], BF16, name=f"m_sb{i}")
                nc.scalar.dma_start(out=t[:], in_=mT3[i])
                m_sb.append(t)

            # ---- V projection: vh[s, c] natural layout, + ones column ----
            # lhsT = xvT chunk [128d, 128s] (stationary), rhs = Wv chunk
            # [128d, 256c] -> psum [128s, 256c]; accumulate over d-chunks.
            # dk-outer so each xv tile is DMA'd once; the 16 per-skt psum
            # accumulators are half a bank each (16 x [128,256]f32 = 8 banks).
            # Two half-phases of 8 skt tiles each (one PSUM bank per skt);
            # xv tiles are streamed again for the second half (cheap DMA).
            vh_sb = [None] * SKT
            with tc.tile_pool(name="ps_v", bufs=1, space="PSUM") as psv_pool:
                for half in range(2):
                    skts = range(half * SKT // 2, (half + 1) * SKT // 2)
                    psv = {
                        skt: psv_pool.tile(
                            [P, COLS], F32, name=f"psv{skt % 8}", tag=f"psv{skt % 8}"
                        )
                        for skt in skts
                    }
                    for dk in range(DK):
                        xt = xpool.tile([P, S], BF16, name="xt", tag="xt", bufs=8)
                        nc.sync.dma_start(out=xt[:], in_=xvT3[dk])
                        for skt in skts:
                            nc.tensor.matmul(
                                psv[skt][:],
                                lhsT=xt[:, skt * P : (skt + 1) * P],
                                rhs=wv_sb[dk][:],
                                start=(dk == 0),
                                stop=(dk == DK - 1),
                            )
                    for skt in skts:
                        vt = vpool.tile([P, NH, HD + 1], BF16, name=f"vh_sb{skt}")
                        nc.vector.tensor_copy(
                            vt[:, :, 0:HD],
                            psv[skt][:].rearrange("p (h d) -> p h d", h=NH),
                        )
                        nc.vector.memset(vt[:, :, HD], 1.0)
                        vh_sb[skt] = vt

            # ---- attention + interleaved output projection ----
            # sqb-outer / head-inner; after each sq block's 4 heads finish,
            # its slice of the output projection runs on psum tiles that
            # share the PV pool tag, so the O-proj of block i overlaps the
            # attention of block i+1 instead of forming a serial tail.
            with (
                tc.tile_pool(name="ps_s", bufs=1, space="PSUM") as ps_s_pool,
                tc.tile_pool(name="ps_pv", bufs=1, space="PSUM") as ps_pv_pool,
            ):
                def emit_oproj(sqb, dcs):
                    q0 = sqb * SQB
                    for dc in dcs:
                        if sqb == NSQB - 1 and dc % 3 != 0:
                            po = ps_s_pool.tile([P, SQB], F32, name="pso2", tag="pss", bufs=2)
                        else:
                            po = ps_pv_pool.tile([P, SQB], F32, name="pso", tag="pso", bufs=1)
                        for sb in range(SQB // 512):
                            for cc in range(COLS // P):
                                nc.tensor.matmul(
                                    po[:, sb * 512 : (sb + 1) * 512],
                                    lhsT=wo_sb[cc][:, dc * P : (dc + 1) * P],
                                    rhs=concat_sb[cc][:, q0 + sb * 512 : q0 + (sb + 1) * 512],
                                    start=(cc == 0),
                                    stop=(cc == COLS // P - 1),
                                )
                        oev = opool.tile([P, SQB], F32, name="oev", tag="oev", bufs=3)
                        if sqb == NSQB - 1 or dc % 2 == 1:
                            nc.scalar.copy(oev[:], po[:])
                        else:
                            nc.vector.tensor_copy(oev[:], po[:])
                        nc.sync.dma_start(out=od3[dc][:, q0 : q0 + SQB], in_=oev[:])

                for sqb in range(NSQB):
                    q0 = sqb * SQB
                    for h in range(NH):
                        ht, hp = h // 2, HD * (h % 2)
                        qT = qhT_sb[("q", ht)]
                        kT = qhT_sb[("k", ht)]
                        pv = ps_pv_pool.tile(
                            [HD + 1, SQB], F32, name="pspv", tag="pspv", bufs=1
                        )
                        # PV emission trails QK by one chunk so that at head
                        # boundaries the next head's QK precedes the last PV in
                        # the PE queue and the exp stream never stalls.
                        def emit_pv(pm_t, skc):
                            for i in range(SQB // 512):
                                nc.tensor.matmul(
                                    pv[:, i * 512 : (i + 1) * 512],
                                    lhsT=vh_sb[skc][:, h, :],
                                    rhs=pm_t[:, i * 512 : (i + 1) * 512],
                                    start=(skc == 0),
                                    stop=(skc == SKT - 1),
                                )

                        pm_prev = None
                        for skc in range(SKT):
                            ss = ps_s_pool.tile(
                                [P, SQB], F32, name="pss", tag="pss", bufs=2
                            )
                            for i in range(SQB // 512):
                                nc.tensor.matmul(
                                    ss[:, i * 512 : (i + 1) * 512],
                                    lhsT=kT[hp : hp + HD, skc * P : (skc + 1) * P],
                                    rhs=qT[hp : hp + HD, q0 + i * 512 : q0 + (i + 1) * 512],
                                    start=True,
                                    stop=True,
                                )
                            if pm_prev is not None:
                                emit_pv(pm_prev, skc - 1)
                            pm = pmpool.tile([P, SQB], BF16, name="pm", tag="pm", bufs=3)
                            nc.scalar.activation(
                                pm[:], ss[:], mybir.ActivationFunctionType.Exp
                            )
                            nc.vector.tensor_mul(
                                pm[:], pm[:], m_sb[skc][:, q0 : q0 + SQB]
                            )
                            pm_prev = pm
                        emit_pv(pm_prev, SKT - 1)
                        # Evacuate the whole PV accumulator to SBUF in one
                        # FD-bound copy (same cost as copying just the denom
                        # row), freeing the psum slot immediately; the rest of
                        # the normalization runs off the SBUF copy.
                        # reciprocal_approx_fast / partition_broadcast only
                        # work on HW for partition-0-based APs, so stage the
                        # denominator row down to partition 0 via a DMA hop.
                        dnc = npool.tile([HD + 1, SQB], F32, name="dnc", tag="dnc", bufs=2)
                        nc.vector.tensor_copy(dnc[:], pv[:])
                        dn0 = npool.tile([1, SQB], F32, name="dn0", tag="dn0", bufs=2)
                        nc.gpsimd.dma_start(out=dn0[:], in_=dnc[HD : HD + 1, :])
                        dnr = npool.tile([1, SQB], F32, name="dnr", tag="dnr", bufs=2)
                        nc.vector.reciprocal_approx_fast(out=dnr[:], in_=dn0[:])
                        rb = npool.tile([HD, SQB], F32, name="rb", tag="rb", bufs=2)
                        nc.gpsimd.partition_broadcast(rb[:], dnr[:])
                        if h % 2 == 0:
                            nc.vector.tensor_mul(
                                concat_sb[ht][0:HD, q0 : q0 + SQB], dnc[0:HD, :], rb[:]
                            )
                        else:
                            tmp = npool.tile([HD, SQB], BF16, name="tmpn", tag="tmpn", bufs=2)
                            nc.vector.tensor_mul(tmp[:], dnc[0:HD, :], rb[:])
                            nc.gpsimd.dma_start(
                                out=concat_sb[ht][HD:P, q0 : q0 + SQB], in_=tmp[:]
                            )
                    # output projection for this sq block (overlaps the
                    # next block's attention via the shared psum slots)
                    emit_oproj(sqb, range(D // P))

    nc.compile()
    return nc


def get_program():
    if "nc" not in _CACHE:
        _CACHE["nc"] = _build_program()
    return _CACHE["nc"]


def make_in_maps(q, k, v, mask, Wq, bq, Wk, bk, Wv, bv, Wo, bo, gate):
    """Host-side sharding: per-core input dict (all numpy)."""
    q, k, v = (np.asarray(a, np.float32) for a in (q, k, v))
    mask = np.asarray(mask)
    Wq, bq, Wk, bk, Wv, bv, Wo, bo, gate = (
        np.asarray(a, np.float32) for a in (Wq, bq, Wk, bk, Wv, bv, Wo, bo, gate)
    )
    scale = 1.0 / np.sqrt(HD)
    xT = {}
    for b in range(B):
        xT[("q", b)] = np.ascontiguousarray(q[b].T).astype(NPBF16)
        xT[("k", b)] = np.ascontiguousarray(k[b].T).astype(NPBF16)
        xT[("v", b)] = np.ascontiguousarray(v[b].T).astype(NPBF16)
        xT[("m", b)] = np.ascontiguousarray(mask[b].T).astype(NPBF16)

    in_maps = []
    for c in range(NCORES):
        b = c // (NCORES // B)
        g = c % (NCORES // B)
        cols = slice(g * COLS, (g + 1) * COLS)
        gate_cols = np.repeat(gate[g * NH : (g + 1) * NH], HD)  # [256]
        in_maps.append(
            {
                "xqT": xT[("q", b)],
                "xkT": xT[("k", b)],
                "xvT": xT[("v", b)],
                "mT": xT[("m", b)],
                # fold the 1/sqrt(hd) score scale into Wq and bq;
                # fold the per-head gate into Wv (bv handled on host)
                "wq": (Wq[:, cols] * scale).astype(NPBF16),
                "wk": Wk[:, cols].astype(NPBF16),
                "wv": (Wv[:, cols] * gate_cols[None, :]).astype(NPBF16),
                "wo": np.ascontiguousarray(Wo[cols, :]).astype(NPBF16),
                "bq": (bq[cols] * scale).astype(np.float32).reshape(COLS, 1),
                "bk": bk[cols].astype(np.float32).reshape(COLS, 1),
            }
        )
    return in_maps


LAST_RESULTS = None


def kernel(q, k, v, mask, Wq, bq, Wk, bk, Wv, bv, Wo, bo, gate, trace=False):
    global LAST_RESULTS
    nc = get_program()
    in_maps = make_in_maps(q, k, v, mask, Wq, bq, Wk, bk, Wv, bv, Wo, bo, gate)
    res = run_bass_kernel_spmd(nc, in_maps, core_ids=list(range(NCORES)), trace=trace)
    LAST_RESULTS = res

    bv_ = np.asarray(bv, np.float32)
    bo_ = np.asarray(bo, np.float32)
    gate_ = np.asarray(gate, np.float32)
    Wo_ = np.asarray(Wo, np.float32)
    # attention rows sum to 1, so the bv term is a constant vector:
    # concat-level constant = repeat(gate, hd) * bv, projected through Wo.
    const = (np.repeat(gate_, HD) * bv_) @ Wo_ + bo_

    out = np.zeros((B, S, D), np.float32)
    for c in range(NCORES):
        b = c // (NCORES // B)
        out[b] += res.results[c]["od"].T
    out += const[None, None, :]
    return out



# revision 65
# speedup vs baseline: 1.1163x; 1.1163x over previous
"""Multi-head attention on 8 Trainium2 NeuronCores.

Problem: B=2, S=2048, D=1024, H=16 heads (head_dim 64), boolean mask,
per-head gate, QKV/out linear projections.

Sharding: core c handles batch b=c//4 and heads 4*(c%4)..4*(c%4)+3.
Each core computes its 4 heads' attention and the partial output
projection (contribution of its 256 concat columns through Wo); the host
sums the 4 partials per batch and adds the constant terms (bo, and the
bv/gate contribution which is constant because attention rows sum to 1).

v2 schedule (PE-bound design; CoreSim cost model):
  - ACT (scalar) engine does ONLY the exp stream (plus the xk DMA that
    completes before the first exp).  All other DMAs live on SP / Pool /
    DVE queues so the 128 x [128,1024] exps are never delayed.
  - scores computed transposed [sk, sq]; exp is a pure ACT pass, mask is
    a multiplicative fp16 tensor_mul on DVE, softmax denominator rides as
    a 65th "ones" column of the PV stationary.
  - V projection is single-pass (skt-outer, xv resident) into 1-bank
    psum scratch slots; evacuation on gpsimd.
  - O projection is cut into [128,512] 1-bank pieces.  sqb0's pieces are
    interleaved into sqb1's attention; sqb1's pieces are split by
    head-pair (cc0 pieces run during h2/h3 attention, cc1 pieces form the
    tail) and written to separate HBM partials summed by the host.
  - normalization (dnc evac / recip / partition_broadcast / final mul) is
    split into 512-halves to shorten the psum-slot hold time and the tail.
  - od partials are written fp16 (host accumulates in fp32).
"""

import sys

if "/opt/trn_rl_repo" not in sys.path:
    sys.path.insert(0, "/opt/trn_rl_repo")

import numpy as np

import concourse.bass as bass
import concourse.bacc as bacc
import concourse.mybir as mybir
import concourse.tile as tile
from concourse.bass_utils import run_bass_kernel_spmd

BF16 = mybir.dt.float16  # fp16: same speed as bf16, 3 more mantissa bits
F32 = mybir.dt.float32
NPBF16 = np.float16

P = 128
B, S, D = 2, 2048, 1024
HEADS, HD = 16, 64
NCORES = 8
NH = HEADS // (NCORES // B)  # heads per core = 4
COLS = NH * HD               # 256 concat columns per core
DK = D // P                  # 8 contraction chunks for the projections
SKT = S // P                 # 16 key chunks
SQB = 1024                   # query block width in the attention loop
NSQB = S // SQB

_CACHE = {}


def _build_program():
    nc = bacc.Bacc("TRN2", debug=False)

    xqT = nc.declare_dram_parameter("xqT", [D, S], BF16, isOutput=False)
    xkT = nc.declare_dram_parameter("xkT", [D, S], BF16, isOutput=False)
    xvT = nc.declare_dram_parameter("xvT", [D, S], BF16, isOutput=False)
    mT = nc.declare_dram_parameter("mT", [S, S], BF16, isOutput=False)
    wq = nc.declare_dram_parameter("wq", [D, COLS], BF16, isOutput=False)
    wk = nc.declare_dram_parameter("wk", [D, COLS], BF16, isOutput=False)
    wv = nc.declare_dram_parameter("wv", [D, COLS], BF16, isOutput=False)
    wo = nc.declare_dram_parameter("wo", [COLS, D], BF16, isOutput=False)
    bq = nc.declare_dram_parameter("bq", [COLS, 1], F32, isOutput=False)
    bk = nc.declare_dram_parameter("bk", [COLS, 1], F32, isOutput=False)
    od = nc.declare_dram_parameter("od", [D, S], BF16, isOutput=True)
    # cc1 (heads 2,3) partial of the LAST sq block, summed on host
    odx = nc.declare_dram_parameter("odx", [D, SQB], BF16, isOutput=True)

    xqT3 = xqT[:].rearrange("(n p) s -> n p s", p=P)
    xkT3 = xkT[:].rearrange("(n p) s -> n p s", p=P)
    xvT3 = xvT[:].rearrange("(n p) s -> n p s", p=P)
    mT3 = mT[:].rearrange("(n p) s -> n p s", p=P)
    wq3 = wq[:].rearrange("(n p) c -> n p c", p=P)
    wk3 = wk[:].rearrange("(n p) c -> n p c", p=P)
    wv3 = wv[:].rearrange("(n p) c -> n p c", p=P)
    wo3 = wo[:].rearrange("(n p) d -> n p d", p=P)
    bq3 = bq[:].rearrange("(n p) o -> n p o", p=P)
    bk3 = bk[:].rearrange("(n p) o -> n p o", p=P)
    od3 = od[:].rearrange("(n p) s -> n p s", p=P)
    odx3 = odx[:].rearrange("(n p) s -> n p s", p=P)

    with tile.TileContext(nc) as tc:
        with (
            tc.tile_pool(name="wpool", bufs=1) as wpool,
            tc.tile_pool(name="qkpool", bufs=1) as qkpool,
            tc.tile_pool(name="vpool", bufs=1) as vpool,
            tc.tile_pool(name="maskpool", bufs=1) as maskpool,
            tc.tile_pool(name="cpool", bufs=1) as cpool,
            tc.tile_pool(name="xpool", bufs=1) as xpool,
            tc.tile_pool(name="pmpool", bufs=1) as pmpool,
            tc.tile_pool(name="npool", bufs=1) as npool,
            tc.tile_pool(name="opool", bufs=1) as opool,
        ):
            # ---------------- DMA issue (t=0), one queue per engine -------
            # SP: weights/biases, then xv (reusing xk's sbuf slots), later od.
            # ACT: xk only (completes before the first exp).
            # DVE: xq only (completes before the first mask-mul).
            # Pool: the 16 mask tiles.
            # each weight tensor is ONE strided DMA into a [128, DK*COLS]
            # tile (8 small DMAs would pay ~0.8us of HWDGE overhead each)
            wk3b = wk[:].rearrange("(n p) c -> p n c", p=P)
            wq3b = wq[:].rearrange("(n p) c -> p n c", p=P)
            wv3b = wv[:].rearrange("(n p) c -> p n c", p=P)
            wo3b = wo[:].rearrange("(n p) d -> p n d", p=P)

            wk_all = wpool.tile([P, DK, COLS], BF16, name="wk_all")
            nc.sync.dma_start(out=wk_all[:], in_=wk3b)
            wk_sb = [wk_all[:, i, :] for i in range(DK)]
            b_sb = {}
            for nm, src in (("bk", bk3), ("bq", bq3)):
                for i in range(COLS // P):
                    t = wpool.tile([P, 1], F32, name=f"{nm}_sb{i}")
                    nc.sync.dma_start(out=t[:], in_=src[i])
                    b_sb[(nm, i)] = t
            wq_all = wpool.tile([P, DK, COLS], BF16, name="wq_all")
            nc.sync.dma_start(out=wq_all[:], in_=wq3b)
            wq_sb = [wq_all[:, i, :] for i in range(DK)]

            # x streams split across queues so neither gates a projection:
            # ACT takes xk0-3 + xq0-3 (done well before the first exp), SP
            # takes xk4-7 + xq4-7 between the weight loads, Pool takes xv
            # (reusing the xk sbuf slots once K-proj drains them) and then
            # the 16 mask tiles.
            xk_t, xq_t, xv_t = [], [], []
            for i in range(DK):
                t = xpool.tile([P, S], BF16, name="xk", tag=f"xk{i}", bufs=1)
                (nc.scalar if i < 4 else nc.sync).dma_start(out=t[:], in_=xkT3[i])
                xk_t.append(t)
            for i in range(DK):
                t = xpool.tile([P, S], BF16, name="xq", tag=f"xq{i}", bufs=1)
                (nc.scalar if i < 4 else nc.sync).dma_start(out=t[:], in_=xqT3[i])
                xq_t.append(t)
            for i in range(DK):
                t = xpool.tile([P, S], BF16, name="xv", tag=f"xv{i}", bufs=1)
                nc.gpsimd.dma_start(out=t[:], in_=xvT3[i])
                xv_t.append(t)

            # mask tiles are resident only for the CURRENT sq block
            # ([128, SQB] halves, one generation per block) — this halves
            # their sbuf footprint so xv gets its own slots above.  The
            # sqb1 generation is DMA'd during sqb0's last head.
            def mask_dma(skc, sqb):
                t = maskpool.tile(
                    [P, SQB], BF16, name=f"m{skc}", tag=f"m{skc}", bufs=1
                )
                nc.gpsimd.dma_start(
                    out=t[:], in_=mT3[skc][:, sqb * SQB : (sqb + 1) * SQB]
                )
                return t

            m_sb = [mask_dma(i, 0) for i in range(SKT)]

            # wv/wo follow the x streams on SP (needed at ~31us / ~85us)
            wv_all = wpool.tile([P, DK, COLS], BF16, name="wv_all")
            nc.sync.dma_start(out=wv_all[:], in_=wv3b)
            wv_sb = [wv_all[:, i, :] for i in range(DK)]
            wo_all = wpool.tile([P, COLS // P, D], BF16, name="wo_all")
            nc.sync.dma_start(out=wo_all[:], in_=wo3b)
            wo_sb = [wo_all[:, i, :] for i in range(COLS // P)]

            # Preload the ACT exp table (~1.3us) while ACT is otherwise idle
            # (right after its xk DMAs), so the first exp doesn't pay it.
            warm = npool.tile([P, 1], F32, name="warm", tag="warm", bufs=1)
            nc.scalar.activation(
                warm[:], b_sb[("bk", 0)][:], mybir.ActivationFunctionType.Exp
            )

            # concat^T (normalized attention outputs, head-major columns)
            concat_sb = [
                cpool.tile([P, S], BF16, name=f"concat_sb{i}")
                for i in range(COLS // P)
            ]

            # ---------------- K then Q projections ----------------------
            # qhT[c, s] = (x @ W + b)^T: lhsT = W chunk [128d, 128c]
            # (stationary), rhs = xT chunk [128d, 512s] -> psum [128c, 512s],
            # accumulated over the 8 d-chunks.  K first (its DMA stream and
            # weights land first), then Q reusing the same psum banks.
            # Only the c0 halves of the K/Q projections (heads 0/1) run
            # before attention; the c1 halves are deferred [128,512]-chunk
            # pieces drained into h0/h1's PE slack (they are needed first
            # by h2, ~35us later).  PE is in-order, so this pulls the first
            # scores ~12us earlier.  Evac+bias runs in [128,512] quarters
            # so the psum bank's next user starts a quarter-latency later.
            qhT_sb = {}
            qhT_sb[("k", 1)] = qkpool.tile([P, S], BF16, name="khT1")
            qhT_sb[("q", 1)] = qkpool.tile([P, S], BF16, name="qhT1")
            with tc.tile_pool(name="ps_proj", bufs=1, space="PSUM") as psp:
                def early_c1_piece(tname, x_t, w_sb, sb):
                    pst = psp.tile([P, 512], F32, name="pc1e", tag="pc1e", bufs=2)
                    for dk in range(DK):
                        nc.tensor.matmul(
                            pst[:],
                            lhsT=w_sb[dk][:, P:COLS],
                            rhs=x_t[dk][:, sb * 512 : (sb + 1) * 512],
                            start=(dk == 0),
                            stop=(dk == DK - 1),
                        )
                    nc.vector.tensor_scalar_add(
                        qhT_sb[(tname, 1)][:, sb * 512 : (sb + 1) * 512],
                        pst[:],
                        b_sb[(f"b{tname}", 1)][:],
                    )

                for tname, x_t, w_sb in (("k", xk_t, wk_sb), ("q", xq_t, wq_sb)):
                    pst = psp.tile([P, S], F32, name=f"psp_{tname}0", tag="psp0")
                    for dk in range(DK):
                        for sb in range(S // 512):
                            nc.tensor.matmul(
                                pst[:, sb * 512 : (sb + 1) * 512],
                                lhsT=w_sb[dk][:, 0:P],
                                rhs=x_t[dk][:, sb * 512 : (sb + 1) * 512],
                                start=(dk == 0),
                                stop=(dk == DK - 1),
                            )
                        # Q-c0's dk matmuls are gated by the xq DMA stream;
                        # fill the PE stalls with K-c1 chunk pieces (their
                        # xk inputs are already resident)
                        if tname == "q" and 1 <= dk <= 4:
                            early_c1_piece("k", xk_t, wk_sb, dk - 1)
                    t = qkpool.tile([P, S], BF16, name=f"{tname}hT0")
                    for qtr in range(4):
                        hs = slice(qtr * 512, (qtr + 1) * 512)
                        nc.vector.tensor_scalar_add(
                            t[:, hs], pst[:, hs], b_sb[(f"b{tname}", 0)][:]
                        )
                    qhT_sb[(tname, 0)] = t

            # ---------------- attention ---------------------------------
            # sqb-outer / head-inner.  V-projection pieces are interleaved
            # into (sqb0, h0); O-projection of sqb_i is interleaved into
            # sqb_{i+1}'s attention; the last sqb's O-proj is split by
            # head-pair with the cc1 half as the (pipelined) tail.
            vh_sb = [None] * SKT

            with (
                tc.tile_pool(name="ps_s", bufs=1, space="PSUM") as ps_s_pool,
                tc.tile_pool(name="ps_pv", bufs=1, space="PSUM") as ps_pv_pool,
            ):
                def emit_vproj(skt):
                    # one V-proj piece: vh[skt] = (xv @ Wv)[skt block] + ones
                    psv = ps_pv_pool.tile(
                        [P, 512], F32, name="psv", tag="scratch", bufs=2
                    )
                    for dk in range(DK):
                        nc.tensor.matmul(
                            psv[:, 0:COLS],
                            lhsT=xv_t[dk][:, skt * P : (skt + 1) * P],
                            rhs=wv_sb[dk][:],
                            start=(dk == 0),
                            stop=(dk == DK - 1),
                        )
                    vt = vpool.tile([P, NH, HD + 1], BF16, name=f"vh_sb{skt}")
                    nc.scalar.copy(
                        vt[:, :, 0:HD],
                        psv[:, 0:COLS].rearrange("p (h d) -> p h d", h=NH),
                    )
                    nc.gpsimd.memset(vt[:, :, HD], 1.0)
                    vh_sb[skt] = vt

                def emit_oproj_piece(sqb, dc, half, ccs, dest3, dq0, evac="pool", dma=None, pool=None):
                    # po[128d, 512sq] = sum_cc wo_cc^T @ concat_cc
                    q0 = sqb * SQB + half * 512
                    if pool == "pss":
                        # tail only: the attention's score psum banks are
                        # dead, reuse them as extra O-proj slots
                        po_w = ps_s_pool.tile([P, SQB], F32, name="pss", tag="pss", bufs=2)
                        po = po_w[:, 0:512]
                    else:
                        po = ps_pv_pool.tile(
                            [P, 512], F32, name="po", tag="scratch", bufs=2
                        )[:]
                    for j, cc in enumerate(ccs):
                        nc.tensor.matmul(
                            po[:],
                            lhsT=wo_sb[cc][:, dc * P : (dc + 1) * P],
                            rhs=concat_sb[cc][:, q0 : q0 + 512],
                            start=(j == 0),
                            stop=(j == len(ccs) - 1),
                        )
                    oev = opool.tile([P, 512], BF16, name="oev", tag="oev", bufs=6)
                    if evac == "act":
                        nc.scalar.copy(oev[:], po)
                    else:
                        nc.vector.tensor_copy(oev[:], po)
                    (dma or nc.sync).dma_start(
                        out=dest3[dc][:, dq0 + half * 512 : dq0 + half * 512 + 512],
                        in_=oev[:],
                    )

                ones64 = npool.tile([HD + 1, HD], F32, name="ones64", tag="ones64", bufs=1)
                nc.vector.memset(ones64[:], 1.0)

                # deferred PE pieces, drained one per skc iteration.  An
                # entry flagged needs_concat (O-proj) may only run once the
                # previous head's concat writes are all emitted.
                pending = []

                def drain(n):
                    k = 0
                    while pending and k < n:
                        fn, needs_concat = pending[0]
                        if needs_concat and carry_norm:
                            break
                        pending.pop(0)
                        fn()
                        k += 1

                def emit_proj_piece(tname, x_t, w_sb, sb):
                    # c1 half of the K/Q projection, one [128,512] chunk
                    pst = ps_pv_pool.tile(
                        [P, 512], F32, name="pc1", tag="scratch", bufs=2
                    )
                    for dk in range(DK):
                        nc.tensor.matmul(
                            pst[:],
                            lhsT=w_sb[dk][:, P:COLS],
                            rhs=x_t[dk][:, sb * 512 : (sb + 1) * 512],
                            start=(dk == 0),
                            stop=(dk == DK - 1),
                        )
                    nc.vector.tensor_scalar_add(
                        qhT_sb[(tname, 1)][:, sb * 512 : (sb + 1) * 512],
                        pst[:],
                        b_sb[(f"b{tname}", 1)][:],
                    )

                # K-c1 ran early (interleaved into Q-c0); Q-c1 chunks are
                # drained into h0/h1's slack (h2 needs chunks 0/1 first)
                for sb in range(4):
                    pending.append(
                        (
                            lambda sb=sb: emit_proj_piece("q", xq_t, wq_sb, sb),
                            False,
                        )
                    )

                def make_norm(pv, h, ht, q0, fast=False):
                    # normalization for head h's pv accumulator, cut into 5
                    # pieces spread over the next head's early iterations:
                    # [dnc evac both halves] [recip0+bcast0] [mul0]
                    # [recip1+bcast1] [mul1].  Evac halves go to Pool+DVE in
                    # parallel so the psum slot frees fast.  The denominator
                    # reciprocal runs in-lane on partition 64 and is
                    # broadcast to partitions 0-63 by a K=1 f32r matmul into
                    # a scratch psum bank.  Odd heads land in concat rows
                    # 64:128 via a gpsimd DMA hop (DVE is lane-locked).
                    dnc = npool.tile(
                        [HD + 1, SQB], F32, name="dnc", tag="dnc", bufs=2
                    )

                    def evac():
                        nc.vector.tensor_copy(dnc[:, 0:512], pv[0][:])
                        nc.vector.tensor_copy(dnc[:, 512:1024], pv[1][:])

                    rbs = {}

                    def recip_piece(half):
                        def fn():
                            sl = slice(half * 512, half * 512 + 512)
                            if fast:
                                # tail-critical: in-lane reciprocal on
                                # partition 64 + fp32 K=1 broadcast matmul
                                # (shorter chain than the hop+broadcast)
                                nc.vector.reciprocal(
                                    out=dnc[HD : HD + 1, sl], in_=dnc[HD : HD + 1, sl]
                                )
                                rb = ps_pv_pool.tile(
                                    [P, 512], F32, name="rbf", tag="scratch", bufs=2
                                )
                                nc.tensor.matmul(
                                    rb[0:HD, :],
                                    lhsT=ones64[HD : HD + 1, :],
                                    rhs=dnc[HD : HD + 1, sl],
                                    start=True,
                                    stop=True,
                                )
                                rbs[half] = rb[0:HD, :]
                                return
                            # hop the denominator row to partition 0 (DVE is
                            # lane-locked; reciprocal_approx_fast and
                            # partition_broadcast are partition-0 ops)
                            dn0 = npool.tile([1, 512], F32, name="dn0", tag="dn0", bufs=1)
                            nc.gpsimd.dma_start(out=dn0[:], in_=dnc[HD : HD + 1, sl])
                            nc.vector.reciprocal_approx_fast(out=dn0[:], in_=dn0[:])
                            rb = npool.tile(
                                [HD, 512], F32, name=f"rb{half}", tag=f"rb{half}", bufs=1
                            )
                            nc.gpsimd.partition_broadcast(rb[:], dn0[:])
                            rbs[half] = rb[:]
                        return fn

                    def mul_piece(half):
                        def fn():
                            sl = slice(half * 512, half * 512 + 512)
                            cs = slice(q0 + half * 512, q0 + half * 512 + 512)
                            if h % 2 == 0:
                                nc.vector.tensor_mul(
                                    concat_sb[ht][0:HD, cs], dnc[0:HD, sl], rbs[half]
                                )
                            else:
                                tmp = npool.tile([HD, 512], BF16, name="tmpn", tag="tmpn", bufs=1)
                                nc.vector.tensor_mul(tmp[:], dnc[0:HD, sl], rbs[half])
                                nc.gpsimd.dma_start(out=concat_sb[ht][HD:P, cs], in_=tmp[:])
                        return fn

                    return [evac, recip_piece(0), mul_piece(0), recip_piece(1), mul_piece(1)]

                # carried finishers from the previous head:
                # [PV(prev,14), PV(prev,15)] then the 5 norm pieces
                carry_pv = []
                carry_norm = []

                m_cur = m_sb
                m_next = [None] * SKT
                for sqb in range(NSQB):
                    q0 = sqb * SQB
                    last_sqb = sqb == NSQB - 1
                    if sqb == 1:
                        m_cur = m_next
                    heads = [0, 1, 3, 2] if last_sqb else [0, 1, 2, 3]
                    for hi, h in enumerate(heads):
                        ht, hp = h // 2, HD * (h % 2)
                        qT = qhT_sb[("q", ht)]
                        kT = qhT_sb[("k", ht)]
                        pv = [
                            ps_pv_pool.tile(
                                [HD + 1, 512], F32, name=f"pspv{i}", tag=f"pspv{i}", bufs=1
                            )
                            for i in range(2)
                        ]

                        def emit_pv(pm_t, skc, pv=pv, h=h):
                            for i in range(SQB // 512):
                                nc.tensor.matmul(
                                    pv[i][:],
                                    lhsT=vh_sb[skc][:, h, :],
                                    rhs=pm_t[:, i * 512 : (i + 1) * 512],
                                    start=(skc == 0),
                                    stop=(skc == SKT - 1),
                                )

                        pms = [None] * SKT
                        for skc in range(SKT):
                            ss = ps_s_pool.tile(
                                [P, SQB], F32, name="pss", tag="pss", bufs=2
                            )
                            for i in range(SQB // 512):
                                nc.tensor.matmul(
                                    ss[:, i * 512 : (i + 1) * 512],
                                    lhsT=kT[hp : hp + HD, skc * P : (skc + 1) * P],
                                    rhs=qT[hp : hp + HD, q0 + i * 512 : q0 + (i + 1) * 512],
                                    start=True,
                                    stop=True,
                                )
                            # PE-order fillers: carried PV flushes of the
                            # previous head at skc 0/1, then this head's PV
                            # trailing by two chunks; V-proj pieces ride in
                            # (sqb0, h0); O-proj pieces drain once concat of
                            # their block is complete (skc>=6 guard).
                            if skc <= 1 and carry_pv:
                                carry_pv.pop(0)()
                            if sqb == 0 and hi == 0:
                                emit_vproj(skc)
                            elif skc % 2 == 1 or len(pending) >= 12:
                                drain(1)
                            if skc >= 2:
                                emit_pv(pms[skc - 2], skc - 2)
                            pm = pmpool.tile([P, SQB], BF16, name="pm", tag="pm", bufs=4)
                            nc.scalar.activation(
                                pm[:], ss[:], mybir.ActivationFunctionType.Exp
                            )
                            nc.vector.tensor_mul(pm[:], pm[:], m_cur[skc][:])
                            pms[skc] = pm
                            # refresh this mask slot with the next block's
                            # half once its last reader is emitted
                            if sqb == 0 and hi == NH - 1 and NSQB > 1:
                                m_next[skc] = mask_dma(skc, 1)
                            # previous head's norm: evac right after the PV
                            # flushes (frees its psum slot), remaining
                            # pieces one per iteration
                            if skc >= 1 and carry_norm:
                                carry_norm.pop(0)()
                        # head end: set up finishers for this head
                        assert not carry_pv and not carry_norm
                        carry_pv = [
                            lambda skc=skc_, f=emit_pv, pms=pms: f(pms[skc], skc)
                            for skc_ in (SKT - 2, SKT - 1)
                        ]
                        carry_norm = make_norm(
                            pv, h, ht, q0, fast=last_sqb and hi == NH - 1
                        )

                        if last_sqb and hi == 1:
                            # cc0 half of the last block's O-proj runs
                            # during the last two heads' attention
                            for dc in range(D // P):
                                for half in range(2):
                                    pending.append(
                                        (
                                            lambda dc=dc, half=half, sqb=sqb: emit_oproj_piece(
                                                sqb, dc, half, [0], od3, sqb * SQB
                                            ),
                                            True,
                                        )
                                    )
                    # end heads
                    if not last_sqb:
                        # whole-block O-proj pieces drained in the next block
                        for dc in range(D // P):
                            for half in range(2):
                                pending.append(
                                    (
                                        lambda sqb=sqb, dc=dc, half=half: emit_oproj_piece(
                                            sqb, dc, half, [0, 1], od3, sqb * SQB
                                        ),
                                        True,
                                    )
                                )
                # final head: flush PVs and run its norm immediately, then
                # the cc1 tail pieces (half-outer so half0 starts after the
                # first mul), written to the odx partial
                for f in carry_pv:
                    f()
                carry_pv = []
                for fn, _ in pending:
                    fn()
                pending = []
                ev, r0, m0, r1, m1 = carry_norm
                ev(); r0(); m0(); r1(); m1()
                for half in range(2):
                    for dc in range(D // P):
                        emit_oproj_piece(
                            NSQB - 1, dc, half, [1], odx3, 0,
                            evac="dve" if dc % 2 else "act",
                        )

    nc.compile()
    return nc


def get_program():
    if "nc" not in _CACHE:
        _CACHE["nc"] = _build_program()
    return _CACHE["nc"]


def make_in_maps(q, k, v, mask, Wq, bq, Wk, bk, Wv, bv, Wo, bo, gate):
    """Host-side sharding: per-core input dict (all numpy)."""
    q, k, v = (np.asarray(a, np.float32) for a in (q, k, v))
    mask = np.asarray(mask)
    Wq, bq, Wk, bk, Wv, bv, Wo, bo, gate = (
        np.asarray(a, np.float32) for a in (Wq, bq, Wk, bk, Wv, bv, Wo, bo, gate)
    )
    scale = 1.0 / np.sqrt(HD)
    xT = {}
    for b in range(B):
        xT[("q", b)] = np.ascontiguousarray(q[b].T).astype(NPBF16)
        xT[("k", b)] = np.ascontiguousarray(k[b].T).astype(NPBF16)
        xT[("v", b)] = np.ascontiguousarray(v[b].T).astype(NPBF16)
        xT[("m", b)] = np.ascontiguousarray(mask[b].T).astype(NPBF16)

    in_maps = []
    for c in range(NCORES):
        b = c // (NCORES // B)
        g = c % (NCORES // B)
        cols = slice(g * COLS, (g + 1) * COLS)
        gate_cols = np.repeat(gate[g * NH : (g + 1) * NH], HD)  # [256]
        in_maps.append(
            {
                "xqT": xT[("q", b)],
                "xkT": xT[("k", b)],
                "xvT": xT[("v", b)],
                "mT": xT[("m", b)],
                # fold the 1/sqrt(hd) score scale into Wq and bq;
                # fold the per-head gate into Wv (bv handled on host)
                "wq": (Wq[:, cols] * scale).astype(NPBF16),
                "wk": Wk[:, cols].astype(NPBF16),
                "wv": (Wv[:, cols] * gate_cols[None, :]).astype(NPBF16),
                "wo": np.ascontiguousarray(Wo[cols, :]).astype(NPBF16),
                "bq": (bq[cols] * scale).astype(np.float32).reshape(COLS, 1),
                "bk": bk[cols].astype(np.float32).reshape(COLS, 1),
            }
        )
    return in_maps


LAST_RESULTS = None


def kernel(q, k, v, mask, Wq, bq, Wk, bk, Wv, bv, Wo, bo, gate, trace=False):
    global LAST_RESULTS
    nc = get_program()
    in_maps = make_in_maps(q, k, v, mask, Wq, bq, Wk, bk, Wv, bv, Wo, bo, gate)
    res = run_bass_kernel_spmd(nc, in_maps, core_ids=list(range(NCORES)), trace=trace)
    LAST_RESULTS = res

    bv_ = np.asarray(bv, np.float32)
    bo_ = np.asarray(bo, np.float32)
    gate_ = np.asarray(gate, np.float32)
    Wo_ = np.asarray(Wo, np.float32)
    # attention rows sum to 1, so the bv term is a constant vector:
    # concat-level constant = repeat(gate, hd) * bv, projected through Wo.
    const = (np.repeat(gate_, HD) * bv_) @ Wo_ + bo_

    out = np.zeros((B, S, D), np.float32)
    for c in range(NCORES):
        b = c // (NCORES // B)
        out[b] += res.results[c]["od"].astype(np.float32).T
        # the last sq block's cc1 contribution is a separate partial
        out[b, (NSQB - 1) * SQB :, :] += res.results[c]["odx"].astype(np.float32).T
    out += const[None, None, :]
    return out


# revision 78
# speedup vs baseline: 1.1292x; 1.0116x over previous
"""Multi-head attention on 8 Trainium2 NeuronCores.

Problem: B=2, S=2048, D=1024, H=16 heads (head_dim 64), boolean mask,
per-head gate, QKV/out linear projections.

Sharding: core c handles batch b=c//4 and heads 4*(c%4)..4*(c%4)+3.
Each core computes its 4 heads' attention and the partial output
projection (contribution of its 256 concat columns through Wo); the host
sums the 4 partials per batch and adds the constant terms (bo, and the
bv/gate contribution which is constant because attention rows sum to 1).

Schedule (PE-bound design, tuned against the CoreSim cost model; all
constructs are neuronxcc/BIR-legal — gpsimd never touches PSUM, no f32r):
  - ACT (scalar) engine runs the 128 x [128,1024] exp stream plus only
    the xk/xq DMAs that complete before the first exp.  Other DMAs live
    on the SP and Pool queues; each weight tensor is one strided DMA.
  - PE is in-order, so emission order IS the schedule.  Only the c0
    halves of the K/Q projections run before attention (first scores at
    ~20us); the c1 halves run as deferred [128,512] pieces inside the
    h0/h1 PE slack (K-c1 interleaves into Q-c0's DMA-gated stalls).
  - scores computed transposed [sk, sq]; exp is a pure ACT pass, mask is
    a multiplicative fp16 tensor_mul on DVE, softmax denominator rides as
    a 65th "ones" column of the PV stationary.  PV trails scores by two
    chunks and the last two PVs + the (5-piece, 512-halved) normalization
    of each head spread across the next head's iterations, so the psum
    slots hand over without stalling PE.
  - V projection is single-pass (skt-outer, xv resident) into 1-bank
    psum scratch slots during h0; evacuation on ACT (slack there).
  - O projection is cut into [128,512] 1-bank pieces drained one per two
    iterations: sqb0's pieces into sqb1's h0/h1, the last block's cc0
    pieces into its h3/h2, and its cc1 pieces form a pipelined tail
    written to the odx partial (summed on host).  Mask tiles are resident
    per-sq-block only (halved footprint funds xv's own sbuf slots).
  - od/odx partials are written fp16 (host accumulates in fp32).
"""

import sys

if "/opt/trn_rl_repo" not in sys.path:
    sys.path.insert(0, "/opt/trn_rl_repo")

import numpy as np

import concourse.bass as bass
import concourse.bacc as bacc
import concourse.mybir as mybir
import concourse.tile as tile
from concourse.bass_utils import run_bass_kernel_spmd

BF16 = mybir.dt.float16  # fp16: same speed as bf16, 3 more mantissa bits
F32 = mybir.dt.float32
NPBF16 = np.float16

P = 128
B, S, D = 2, 2048, 1024
HEADS, HD = 16, 64
NCORES = 8
NH = HEADS // (NCORES // B)  # heads per core = 4
COLS = NH * HD               # 256 concat columns per core
DK = D // P                  # 8 contraction chunks for the projections
SKT = S // P                 # 16 key chunks
SQB = 1024                   # query block width in the attention loop
NSQB = S // SQB

_CACHE = {}


def _build_program():
    nc = bacc.Bacc("TRN2", debug=False)

    xqT = nc.declare_dram_parameter("xqT", [D, S], BF16, isOutput=False)
    xkT = nc.declare_dram_parameter("xkT", [D, S], BF16, isOutput=False)
    xvT = nc.declare_dram_parameter("xvT", [D, S], BF16, isOutput=False)
    mT = nc.declare_dram_parameter("mT", [S, S], BF16, isOutput=False)
    wq = nc.declare_dram_parameter("wq", [D, COLS], BF16, isOutput=False)
    wk = nc.declare_dram_parameter("wk", [D, COLS], BF16, isOutput=False)
    wv = nc.declare_dram_parameter("wv", [D, COLS], BF16, isOutput=False)
    wo = nc.declare_dram_parameter("wo", [COLS, D], BF16, isOutput=False)
    bq = nc.declare_dram_parameter("bq", [COLS, 1], F32, isOutput=False)
    bk = nc.declare_dram_parameter("bk", [COLS, 1], F32, isOutput=False)
    od = nc.declare_dram_parameter("od", [D, S], BF16, isOutput=True)
    # cc1 (heads 2,3) partial of the LAST sq block, summed on host
    odx = nc.declare_dram_parameter("odx", [D, SQB], BF16, isOutput=True)

    xqT3 = xqT[:].rearrange("(n p) s -> n p s", p=P)
    xkT3 = xkT[:].rearrange("(n p) s -> n p s", p=P)
    xvT3 = xvT[:].rearrange("(n p) s -> n p s", p=P)
    mT3 = mT[:].rearrange("(n p) s -> n p s", p=P)
    wq3 = wq[:].rearrange("(n p) c -> n p c", p=P)
    wk3 = wk[:].rearrange("(n p) c -> n p c", p=P)
    wv3 = wv[:].rearrange("(n p) c -> n p c", p=P)
    wo3 = wo[:].rearrange("(n p) d -> n p d", p=P)
    bq3 = bq[:].rearrange("(n p) o -> n p o", p=P)
    bk3 = bk[:].rearrange("(n p) o -> n p o", p=P)
    od3 = od[:].rearrange("(n p) s -> n p s", p=P)
    odx3 = odx[:].rearrange("(n p) s -> n p s", p=P)

    with tile.TileContext(nc) as tc:
        with (
            tc.tile_pool(name="wpool", bufs=1) as wpool,
            tc.tile_pool(name="qkpool", bufs=1) as qkpool,
            tc.tile_pool(name="vpool", bufs=1) as vpool,
            tc.tile_pool(name="maskpool", bufs=1) as maskpool,
            tc.tile_pool(name="cpool", bufs=1) as cpool,
            tc.tile_pool(name="xpool", bufs=1) as xpool,
            tc.tile_pool(name="pmpool", bufs=1) as pmpool,
            tc.tile_pool(name="npool", bufs=1) as npool,
            tc.tile_pool(name="opool", bufs=1) as opool,
        ):
            # ---------------- DMA issue (t=0), one queue per engine -------
            # SP: weights/biases, then xv (reusing xk's sbuf slots), later od.
            # ACT: xk only (completes before the first exp).
            # DVE: xq only (completes before the first mask-mul).
            # Pool: the 16 mask tiles.
            # each weight tensor is ONE strided DMA into a [128, DK*COLS]
            # tile (8 small DMAs would pay ~0.8us of HWDGE overhead each)
            wk3b = wk[:].rearrange("(n p) c -> p n c", p=P)
            wq3b = wq[:].rearrange("(n p) c -> p n c", p=P)
            wv3b = wv[:].rearrange("(n p) c -> p n c", p=P)
            wo3b = wo[:].rearrange("(n p) d -> p n d", p=P)

            wk_all = wpool.tile([P, DK, COLS], BF16, name="wk_all")
            nc.sync.dma_start(out=wk_all[:], in_=wk3b)
            wk_sb = [wk_all[:, i, :] for i in range(DK)]
            b_sb = {}
            for nm, src in (("bk", bk3), ("bq", bq3)):
                for i in range(COLS // P):
                    t = wpool.tile([P, 1], F32, name=f"{nm}_sb{i}")
                    nc.sync.dma_start(out=t[:], in_=src[i])
                    b_sb[(nm, i)] = t
            wq_all = wpool.tile([P, DK, COLS], BF16, name="wq_all")
            nc.sync.dma_start(out=wq_all[:], in_=wq3b)
            wq_sb = [wq_all[:, i, :] for i in range(DK)]

            # x streams split across queues so neither gates a projection:
            # ACT takes xk0-3 + xq0-3 (done well before the first exp), SP
            # takes xk4-7 + xq4-7 between the weight loads, Pool takes xv
            # (reusing the xk sbuf slots once K-proj drains them) and then
            # the 16 mask tiles.
            xk_t, xq_t, xv_t = [], [], []
            for i in range(DK):
                t = xpool.tile([P, S], BF16, name="xk", tag=f"xk{i}", bufs=1)
                # xk0 gates the very first matmul: the Pool/SWDGE queue
                # delivers it ~0.9us sooner than ACT's HWDGE path
                eng = nc.gpsimd if i == 0 else (nc.scalar if i < 4 else nc.sync)
                eng.dma_start(out=t[:], in_=xkT3[i])
                xk_t.append(t)
            for i in range(DK):
                t = xpool.tile([P, S], BF16, name="xq", tag=f"xq{i}", bufs=1)
                (nc.scalar if i < 4 else nc.sync).dma_start(out=t[:], in_=xqT3[i])
                xq_t.append(t)
            for i in range(DK):
                t = xpool.tile([P, S], BF16, name="xv", tag=f"xv{i}", bufs=1)
                nc.gpsimd.dma_start(out=t[:], in_=xvT3[i])
                xv_t.append(t)

            # mask tiles are resident only for the CURRENT sq block
            # ([128, SQB] halves, one generation per block) — this halves
            # their sbuf footprint so xv gets its own slots above.  The
            # sqb1 generation is DMA'd during sqb0's last head.
            def mask_dma(skc, sqb):
                t = maskpool.tile(
                    [P, SQB], BF16, name=f"m{skc}", tag=f"m{skc}", bufs=1
                )
                nc.gpsimd.dma_start(
                    out=t[:], in_=mT3[skc][:, sqb * SQB : (sqb + 1) * SQB]
                )
                return t

            m_sb = [mask_dma(i, 0) for i in range(SKT)]

            # wv/wo follow the x streams on SP (needed at ~31us / ~85us)
            wv_all = wpool.tile([P, DK, COLS], BF16, name="wv_all")
            nc.sync.dma_start(out=wv_all[:], in_=wv3b)
            wv_sb = [wv_all[:, i, :] for i in range(DK)]
            wo_all = wpool.tile([P, COLS // P, D], BF16, name="wo_all")
            nc.sync.dma_start(out=wo_all[:], in_=wo3b)
            wo_sb = [wo_all[:, i, :] for i in range(COLS // P)]

            # Preload the ACT exp table (~1.3us) while ACT is otherwise idle
            # (right after its xk DMAs), so the first exp doesn't pay it.
            warm = npool.tile([P, 1], F32, name="warm", tag="warm", bufs=1)
            nc.scalar.activation(
                warm[:], b_sb[("bk", 0)][:], mybir.ActivationFunctionType.Exp
            )

            # concat^T (normalized attention outputs, head-major columns)
            concat_sb = [
                cpool.tile([P, S], BF16, name=f"concat_sb{i}")
                for i in range(COLS // P)
            ]

            # ---------------- K then Q projections ----------------------
            # qhT[c, s] = (x @ W + b)^T: lhsT = W chunk [128d, 128c]
            # (stationary), rhs = xT chunk [128d, 512s] -> psum [128c, 512s],
            # accumulated over the 8 d-chunks.  K first (its DMA stream and
            # weights land first), then Q reusing the same psum banks.
            # Only the c0 halves of the K/Q projections (heads 0/1) run
            # before attention; the c1 halves are deferred [128,512]-chunk
            # pieces drained into h0/h1's PE slack (they are needed first
            # by h2, ~35us later).  PE is in-order, so this pulls the first
            # scores ~12us earlier.  Evac+bias runs in [128,512] quarters
            # so the psum bank's next user starts a quarter-latency later.
            qhT_sb = {}
            qhT_sb[("k", 1)] = qkpool.tile([P, S], BF16, name="khT1")
            qhT_sb[("q", 1)] = qkpool.tile([P, S], BF16, name="qhT1")
            with tc.tile_pool(name="ps_proj", bufs=1, space="PSUM") as psp:
                def early_c1_piece(tname, x_t, w_sb, sb):
                    pst = psp.tile([P, 512], F32, name="pc1e", tag="pc1e", bufs=2)
                    for dk in range(DK):
                        nc.tensor.matmul(
                            pst[:],
                            lhsT=w_sb[dk][:, P:COLS],
                            rhs=x_t[dk][:, sb * 512 : (sb + 1) * 512],
                            start=(dk == 0),
                            stop=(dk == DK - 1),
                        )
                    nc.vector.tensor_scalar_add(
                        qhT_sb[(tname, 1)][:, sb * 512 : (sb + 1) * 512],
                        pst[:],
                        b_sb[(f"b{tname}", 1)][:],
                    )

                for tname, x_t, w_sb in (("k", xk_t, wk_sb), ("q", xq_t, wq_sb)):
                    pst = psp.tile([P, S], F32, name=f"psp_{tname}0", tag="psp0")
                    for dk in range(DK):
                        for sb in range(S // 512):
                            nc.tensor.matmul(
                                pst[:, sb * 512 : (sb + 1) * 512],
                                lhsT=w_sb[dk][:, 0:P],
                                rhs=x_t[dk][:, sb * 512 : (sb + 1) * 512],
                                start=(dk == 0),
                                stop=(dk == DK - 1),
                            )
                        # Q-c0's dk matmuls are gated by the xq DMA stream;
                        # fill the PE stalls with K-c1 chunk pieces (their
                        # xk inputs are already resident)
                        if tname == "q" and 1 <= dk <= 4:
                            early_c1_piece("k", xk_t, wk_sb, dk - 1)
                    t = qkpool.tile([P, S], BF16, name=f"{tname}hT0")
                    for qtr in range(4):
                        hs = slice(qtr * 512, (qtr + 1) * 512)
                        nc.vector.tensor_scalar_add(
                            t[:, hs], pst[:, hs], b_sb[(f"b{tname}", 0)][:]
                        )
                    qhT_sb[(tname, 0)] = t

            # ---------------- attention ---------------------------------
            # sqb-outer / head-inner.  V-projection pieces are interleaved
            # into (sqb0, h0); O-projection of sqb_i is interleaved into
            # sqb_{i+1}'s attention; the last sqb's O-proj is split by
            # head-pair with the cc1 half as the (pipelined) tail.
            vh_sb = [None] * SKT

            with (
                tc.tile_pool(name="ps_s", bufs=1, space="PSUM") as ps_s_pool,
                tc.tile_pool(name="ps_pv", bufs=1, space="PSUM") as ps_pv_pool,
            ):
                def emit_vproj(skt):
                    # one V-proj piece: vh[skt] = (xv @ Wv)[skt block] + ones
                    psv = ps_pv_pool.tile(
                        [P, 512], F32, name="psv", tag="scratch", bufs=2
                    )
                    for dk in range(DK):
                        nc.tensor.matmul(
                            psv[:, 0:COLS],
                            lhsT=xv_t[dk][:, skt * P : (skt + 1) * P],
                            rhs=wv_sb[dk][:],
                            start=(dk == 0),
                            stop=(dk == DK - 1),
                        )
                    vt = vpool.tile([P, NH, HD + 1], BF16, name=f"vh_sb{skt}")
                    nc.scalar.copy(
                        vt[:, :, 0:HD],
                        psv[:, 0:COLS].rearrange("p (h d) -> p h d", h=NH),
                    )
                    nc.gpsimd.memset(vt[:, :, HD], 1.0)
                    vh_sb[skt] = vt

                def emit_oproj_piece(sqb, dc, half, ccs, dest3, dq0, evac="pool", dma=None, pool=None):
                    # po[128d, 512sq] = sum_cc wo_cc^T @ concat_cc
                    q0 = sqb * SQB + half * 512
                    if pool == "pss":
                        # tail only: the attention's score psum banks are
                        # dead, reuse them as extra O-proj slots
                        po_w = ps_s_pool.tile([P, SQB], F32, name="pss", tag="pss", bufs=2)
                        po = po_w[:, 0:512]
                    else:
                        po = ps_pv_pool.tile(
                            [P, 512], F32, name="po", tag="scratch", bufs=2
                        )[:]
                    for j, cc in enumerate(ccs):
                        nc.tensor.matmul(
                            po[:],
                            lhsT=wo_sb[cc][:, dc * P : (dc + 1) * P],
                            rhs=concat_sb[cc][:, q0 : q0 + 512],
                            start=(j == 0),
                            stop=(j == len(ccs) - 1),
                        )
                    oev = opool.tile([P, 512], BF16, name="oev", tag="oev", bufs=6)
                    if evac == "act":
                        nc.scalar.copy(oev[:], po)
                    else:
                        nc.vector.tensor_copy(oev[:], po)
                    (dma or nc.sync).dma_start(
                        out=dest3[dc][:, dq0 + half * 512 : dq0 + half * 512 + 512],
                        in_=oev[:],
                    )

                ones64 = npool.tile([HD + 1, HD], F32, name="ones64", tag="ones64", bufs=1)
                nc.vector.memset(ones64[:], 1.0)

                # deferred PE pieces, drained one per skc iteration.  An
                # entry flagged needs_concat (O-proj) may only run once the
                # previous head's concat writes are all emitted.
                pending = []

                def drain(n):
                    k = 0
                    while pending and k < n:
                        fn, needs_concat = pending[0]
                        if needs_concat and carry_norm:
                            break
                        pending.pop(0)
                        fn()
                        k += 1

                def emit_proj_piece(tname, x_t, w_sb, sb):
                    # c1 half of the K/Q projection, one [128,512] chunk
                    pst = ps_pv_pool.tile(
                        [P, 512], F32, name="pc1", tag="scratch", bufs=2
                    )
                    for dk in range(DK):
                        nc.tensor.matmul(
                            pst[:],
                            lhsT=w_sb[dk][:, P:COLS],
                            rhs=x_t[dk][:, sb * 512 : (sb + 1) * 512],
                            start=(dk == 0),
                            stop=(dk == DK - 1),
                        )
                    nc.vector.tensor_scalar_add(
                        qhT_sb[(tname, 1)][:, sb * 512 : (sb + 1) * 512],
                        pst[:],
                        b_sb[(f"b{tname}", 1)][:],
                    )

                # K-c1 ran early (interleaved into Q-c0); Q-c1 chunks are
                # drained into h0/h1's slack (h2 needs chunks 0/1 first)
                for sb in range(4):
                    pending.append(
                        (
                            lambda sb=sb: emit_proj_piece("q", xq_t, wq_sb, sb),
                            False,
                        )
                    )

                def make_norm(pv, h, ht, q0, fast=False):
                    # normalization for head h's pv accumulator, cut into 5
                    # pieces spread over the next head's early iterations:
                    # [dnc evac both halves] [recip0+bcast0] [mul0]
                    # [recip1+bcast1] [mul1].  Evac halves go to Pool+DVE in
                    # parallel so the psum slot frees fast.  The denominator
                    # reciprocal runs in-lane on partition 64 and is
                    # broadcast to partitions 0-63 by a K=1 f32r matmul into
                    # a scratch psum bank.  Odd heads land in concat rows
                    # 64:128 via a gpsimd DMA hop (DVE is lane-locked).
                    dnc = npool.tile(
                        [HD + 1, SQB], F32, name="dnc", tag="dnc", bufs=2
                    )

                    def evac_half(i):
                        def fn():
                            nc.vector.tensor_copy(
                                dnc[:, i * 512 : (i + 1) * 512], pv[i][:]
                            )
                        return fn

                    def evac():
                        evac_half(0)()
                        evac_half(1)()

                    rbs = {}

                    def recip_piece(half):
                        def fn():
                            sl = slice(half * 512, half * 512 + 512)
                            if fast:
                                # tail-critical: in-lane reciprocal on
                                # partition 64 + fp32 K=1 broadcast matmul
                                # (shorter chain than the hop+broadcast)
                                nc.vector.reciprocal(
                                    out=dnc[HD : HD + 1, sl], in_=dnc[HD : HD + 1, sl]
                                )
                                rb = ps_pv_pool.tile(
                                    [P, 512], F32, name="rbf", tag="scratch", bufs=2
                                )
                                nc.tensor.matmul(
                                    rb[0:HD, :],
                                    lhsT=ones64[HD : HD + 1, :],
                                    rhs=dnc[HD : HD + 1, sl],
                                    start=True,
                                    stop=True,
                                )
                                rbs[half] = rb[0:HD, :]
                                return
                            # hop the denominator row to partition 0 (DVE is
                            # lane-locked; reciprocal_approx_fast and
                            # partition_broadcast are partition-0 ops)
                            dn0 = npool.tile([1, 512], F32, name="dn0", tag="dn0", bufs=1)
                            nc.gpsimd.dma_start(out=dn0[:], in_=dnc[HD : HD + 1, sl])
                            nc.vector.reciprocal_approx_fast(out=dn0[:], in_=dn0[:])
                            rb = npool.tile(
                                [HD, 512], F32, name=f"rb{half}", tag=f"rb{half}", bufs=1
                            )
                            nc.gpsimd.partition_broadcast(rb[:], dn0[:])
                            rbs[half] = rb[:]
                        return fn

                    def mul_piece(half):
                        def fn():
                            sl = slice(half * 512, half * 512 + 512)
                            cs = slice(q0 + half * 512, q0 + half * 512 + 512)
                            if h % 2 == 0:
                                nc.vector.tensor_mul(
                                    concat_sb[ht][0:HD, cs], dnc[0:HD, sl], rbs[half]
                                )
                            else:
                                tmp = npool.tile([HD, 512], BF16, name="tmpn", tag="tmpn", bufs=1)
                                nc.vector.tensor_mul(tmp[:], dnc[0:HD, sl], rbs[half])
                                nc.gpsimd.dma_start(out=concat_sb[ht][HD:P, cs], in_=tmp[:])
                        return fn

                    if fast:
                        # tail: per-half chains so the first O-proj pieces
                        # start after half0's mul instead of the full chain
                        return [
                            evac_half(0), recip_piece(0), mul_piece(0),
                            evac_half(1), recip_piece(1), mul_piece(1),
                        ]
                    return [evac, recip_piece(0), mul_piece(0), recip_piece(1), mul_piece(1)]

                # carried finishers from the previous head:
                # [PV(prev,14), PV(prev,15)] then the 5 norm pieces
                carry_pv = []
                carry_norm = []

                m_cur = m_sb
                m_next = [None] * SKT
                for sqb in range(NSQB):
                    q0 = sqb * SQB
                    last_sqb = sqb == NSQB - 1
                    if sqb == 1:
                        m_cur = m_next
                    heads = [0, 1, 3, 2] if last_sqb else [0, 1, 2, 3]
                    for hi, h in enumerate(heads):
                        ht, hp = h // 2, HD * (h % 2)
                        qT = qhT_sb[("q", ht)]
                        kT = qhT_sb[("k", ht)]
                        pv = [
                            ps_pv_pool.tile(
                                [HD + 1, 512], F32, name=f"pspv{i}", tag=f"pspv{i}", bufs=1
                            )
                            for i in range(2)
                        ]

                        def emit_pv(pm_t, skc, pv=pv, h=h):
                            for i in range(SQB // 512):
                                nc.tensor.matmul(
                                    pv[i][:],
                                    lhsT=vh_sb[skc][:, h, :],
                                    rhs=pm_t[:, i * 512 : (i + 1) * 512],
                                    start=(skc == 0),
                                    stop=(skc == SKT - 1),
                                )

                        pms = [None] * SKT
                        for skc in range(SKT):
                            ss = ps_s_pool.tile(
                                [P, SQB], F32, name="pss", tag="pss", bufs=2
                            )
                            for i in range(SQB // 512):
                                nc.tensor.matmul(
                                    ss[:, i * 512 : (i + 1) * 512],
                                    lhsT=kT[hp : hp + HD, skc * P : (skc + 1) * P],
                                    rhs=qT[hp : hp + HD, q0 + i * 512 : q0 + (i + 1) * 512],
                                    start=True,
                                    stop=True,
                                )
                            # PE-order fillers: carried PV flushes of the
                            # previous head at skc 0/1, then this head's PV
                            # trailing by two chunks; V-proj pieces ride in
                            # (sqb0, h0); O-proj pieces drain once concat of
                            # their block is complete (skc>=6 guard).
                            if skc <= 1 and carry_pv:
                                carry_pv.pop(0)()
                            if sqb == 0 and hi == 0:
                                emit_vproj(skc)
                            elif skc % 2 == 1 or len(pending) >= 12:
                                drain(1)
                            if skc >= 2:
                                emit_pv(pms[skc - 2], skc - 2)
                            pm = pmpool.tile([P, SQB], BF16, name="pm", tag="pm", bufs=4)
                            nc.scalar.activation(
                                pm[:], ss[:], mybir.ActivationFunctionType.Exp
                            )
                            # every 4th mask-mul rides on the idle gpsimd
                            # engine to relieve DVE pacing (pm is consumed
                            # two iterations later, so the slower engine's
                            # latency is hidden)
                            meng = nc.gpsimd if skc % 4 == 3 else nc.vector
                            meng.tensor_mul(pm[:], pm[:], m_cur[skc][:])
                            pms[skc] = pm
                            # refresh this mask slot with the next block's
                            # half once its last reader is emitted
                            if sqb == 0 and hi == NH - 1 and NSQB > 1:
                                m_next[skc] = mask_dma(skc, 1)
                            # previous head's norm: evac right after the PV
                            # flushes (frees its psum slot), remaining
                            # pieces one per iteration
                            if skc >= 1 and carry_norm:
                                carry_norm.pop(0)()
                        # head end: set up finishers for this head
                        assert not carry_pv and not carry_norm
                        carry_pv = [
                            lambda skc=skc_, f=emit_pv, pms=pms: f(pms[skc], skc)
                            for skc_ in (SKT - 2, SKT - 1)
                        ]
                        carry_norm = make_norm(
                            pv, h, ht, q0, fast=last_sqb and hi == NH - 1
                        )

                        if last_sqb and hi == 1:
                            # cc0 half of the last block's O-proj runs
                            # during the last two heads' attention
                            for dc in range(D // P):
                                for half in range(2):
                                    pending.append(
                                        (
                                            lambda dc=dc, half=half, sqb=sqb: emit_oproj_piece(
                                                sqb, dc, half, [0], od3, sqb * SQB
                                            ),
                                            True,
                                        )
                                    )
                    # end heads
                    if not last_sqb:
                        # whole-block O-proj pieces drained in the next block
                        for dc in range(D // P):
                            for half in range(2):
                                pending.append(
                                    (
                                        lambda sqb=sqb, dc=dc, half=half: emit_oproj_piece(
                                            sqb, dc, half, [0, 1], od3, sqb * SQB
                                        ),
                                        True,
                                    )
                                )
                # final head: flush PVs and run its norm immediately, then
                # the cc1 tail pieces (half-outer so half0 starts after the
                # first mul), written to the odx partial
                for f in carry_pv:
                    f()
                carry_pv = []
                for fn, _ in pending:
                    fn()
                pending = []
                ev0, r0, m0, ev1, r1, m1 = carry_norm
                ev0(); r0(); m0(); ev1(); r1()
                for dc in range(D // P):
                    if dc == 2:
                        m1()
                    emit_oproj_piece(
                        NSQB - 1, dc, 0, [1], odx3, 0,
                        evac="dve" if dc % 2 else "act",
                        pool="pss" if dc % 2 else None,
                    )
                for dc in range(D // P):
                    emit_oproj_piece(
                        NSQB - 1, dc, 1, [1], odx3, 0,
                        evac="dve" if dc % 2 else "act",
                        pool="pss" if dc % 2 else None,
                    )

    nc.compile()
    return nc


def get_program():
    if "nc" not in _CACHE:
        _CACHE["nc"] = _build_program()
    return _CACHE["nc"]


def make_in_maps(q, k, v, mask, Wq, bq, Wk, bk, Wv, bv, Wo, bo, gate):
    """Host-side sharding: per-core input dict (all numpy)."""
    q, k, v = (np.asarray(a, np.float32) for a in (q, k, v))
    mask = np.asarray(mask)
    Wq, bq, Wk, bk, Wv, bv, Wo, bo, gate = (
        np.asarray(a, np.float32) for a in (Wq, bq, Wk, bk, Wv, bv, Wo, bo, gate)
    )
    scale = 1.0 / np.sqrt(HD)
    xT = {}
    for b in range(B):
        xT[("q", b)] = np.ascontiguousarray(q[b].T).astype(NPBF16)
        xT[("k", b)] = np.ascontiguousarray(k[b].T).astype(NPBF16)
        xT[("v", b)] = np.ascontiguousarray(v[b].T).astype(NPBF16)
        xT[("m", b)] = np.ascontiguousarray(mask[b].T).astype(NPBF16)

    in_maps = []
    for c in range(NCORES):
        b = c // (NCORES // B)
        g = c % (NCORES // B)
        cols = slice(g * COLS, (g + 1) * COLS)
        gate_cols = np.repeat(gate[g * NH : (g + 1) * NH], HD)  # [256]
        in_maps.append(
            {
                "xqT": xT[("q", b)],
                "xkT": xT[("k", b)],
                "xvT": xT[("v", b)],
                "mT": xT[("m", b)],
                # fold the 1/sqrt(hd) score scale into Wq and bq;
                # fold the per-head gate into Wv (bv handled on host)
                "wq": (Wq[:, cols] * scale).astype(NPBF16),
                "wk": Wk[:, cols].astype(NPBF16),
                "wv": (Wv[:, cols] * gate_cols[None, :]).astype(NPBF16),
                "wo": np.ascontiguousarray(Wo[cols, :]).astype(NPBF16),
                "bq": (bq[cols] * scale).astype(np.float32).reshape(COLS, 1),
                "bk": bk[cols].astype(np.float32).reshape(COLS, 1),
            }
        )
    return in_maps


LAST_RESULTS = None


def kernel(q, k, v, mask, Wq, bq, Wk, bk, Wv, bv, Wo, bo, gate, trace=False):
    global LAST_RESULTS
    nc = get_program()
    in_maps = make_in_maps(q, k, v, mask, Wq, bq, Wk, bk, Wv, bv, Wo, bo, gate)
    res = run_bass_kernel_spmd(nc, in_maps, core_ids=list(range(NCORES)), trace=trace)
    LAST_RESULTS = res

    bv_ = np.asarray(bv, np.float32)
    bo_ = np.asarray(bo, np.float32)
    gate_ = np.asarray(gate, np.float32)
    Wo_ = np.asarray(Wo, np.float32)
    # attention rows sum to 1, so the bv term is a constant vector:
    # concat-level constant = repeat(gate, hd) * bv, projected through Wo.
    const = (np.repeat(gate_, HD) * bv_) @ Wo_ + bo_

    out = np.zeros((B, S, D), np.float32)
    for c in range(NCORES):
        b = c // (NCORES // B)
        out[b] += res.results[c]["od"].astype(np.float32).T
        # the last sq block's cc1 contribution is a separate partial
        out[b, (NSQB - 1) * SQB :, :] += res.results[c]["odx"].astype(np.float32).T
    out += const[None, None, :]
    return out


# revision 87
# speedup vs baseline: 1.1448x; 1.0137x over previous
"""Multi-head attention on 8 Trainium2 NeuronCores.

Problem: B=2, S=2048, D=1024, H=16 heads (head_dim 64), boolean mask,
per-head gate, QKV/out linear projections.

Sharding: core c handles batch b=c//4 and heads 4*(c%4)..4*(c%4)+3.
Each core computes its 4 heads' attention and the partial output
projection (contribution of its 256 concat columns through Wo); the host
sums the 4 partials per batch and adds the constant terms (bo, and the
bv/gate contribution which is constant because attention rows sum to 1).

Schedule (PE-bound design, tuned against the CoreSim cost model; all
constructs are neuronxcc/BIR-legal — gpsimd never touches PSUM, no f32r):
  - ACT (scalar) engine runs the 128 x [128,1024] exp stream plus only
    the xk/xq DMAs that complete before the first exp.  Other DMAs live
    on the SP and Pool queues; each weight tensor is one strided DMA.
  - PE is in-order, so emission order IS the schedule.  Only the c0
    halves of the K/Q projections run before attention (first scores at
    ~20us); the c1 halves run as deferred [128,512] pieces inside the
    h0/h1 PE slack (K-c1 interleaves into Q-c0's DMA-gated stalls).
  - scores computed transposed [sk, sq]; exp is a pure ACT pass, mask is
    a multiplicative fp16 tensor_mul on DVE, softmax denominator rides as
    a 65th "ones" column of the PV stationary.  PV trails scores by two
    chunks and the last two PVs + the (5-piece, 512-halved) normalization
    of each head spread across the next head's iterations, so the psum
    slots hand over without stalling PE.
  - V projection is single-pass (skt-outer, xv resident) into 1-bank
    psum scratch slots during h0; evacuation on ACT (slack there).
  - O projection is cut into [128,512] 1-bank pieces drained one per two
    iterations: sqb0's pieces into sqb1's h0/h1, the last block's cc0
    pieces into its h3/h2, and its cc1 pieces form a pipelined tail
    written to the odx partial (summed on host).  Mask tiles are resident
    per-sq-block only (halved footprint funds xv's own sbuf slots).
  - od/odx partials are written fp16 (host accumulates in fp32).
"""

import sys

if "/opt/trn_rl_repo" not in sys.path:
    sys.path.insert(0, "/opt/trn_rl_repo")

import numpy as np

import concourse.bass as bass
import concourse.bacc as bacc
import concourse.mybir as mybir
import concourse.tile as tile
from concourse.bass_utils import run_bass_kernel_spmd

BF16 = mybir.dt.float16  # fp16: same speed as bf16, 3 more mantissa bits
F32 = mybir.dt.float32
NPBF16 = np.float16

P = 128
B, S, D = 2, 2048, 1024
HEADS, HD = 16, 64
NCORES = 8
NH = HEADS // (NCORES // B)  # heads per core = 4
COLS = NH * HD               # 256 concat columns per core
DK = D // P                  # 8 contraction chunks for the projections
SKT = S // P                 # 16 key chunks
SQB = 1024                   # query block width in the attention loop
NSQB = S // SQB

_CACHE = {}


def _build_program():
    nc = bacc.Bacc("TRN2", debug=False)

    xqT = nc.declare_dram_parameter("xqT", [D, S], BF16, isOutput=False)
    xkT = nc.declare_dram_parameter("xkT", [D, S], BF16, isOutput=False)
    xvT = nc.declare_dram_parameter("xvT", [D, S], BF16, isOutput=False)
    mT = nc.declare_dram_parameter("mT", [S, S], BF16, isOutput=False)
    wq = nc.declare_dram_parameter("wq", [D, COLS], BF16, isOutput=False)
    wk = nc.declare_dram_parameter("wk", [D, COLS], BF16, isOutput=False)
    wv = nc.declare_dram_parameter("wv", [D, COLS], BF16, isOutput=False)
    wo = nc.declare_dram_parameter("wo", [COLS, D], BF16, isOutput=False)
    bq = nc.declare_dram_parameter("bq", [COLS, 1], F32, isOutput=False)
    bk = nc.declare_dram_parameter("bk", [COLS, 1], F32, isOutput=False)
    od = nc.declare_dram_parameter("od", [D, S], BF16, isOutput=True)
    # cc1 (heads 2,3) partial of the LAST sq block, summed on host
    odx = nc.declare_dram_parameter("odx", [D, SQB], BF16, isOutput=True)

    xqT3 = xqT[:].rearrange("(n p) s -> n p s", p=P)
    xkT3 = xkT[:].rearrange("(n p) s -> n p s", p=P)
    xvT3 = xvT[:].rearrange("(n p) s -> n p s", p=P)
    mT3 = mT[:].rearrange("(n p) s -> n p s", p=P)
    wq3 = wq[:].rearrange("(n p) c -> n p c", p=P)
    wk3 = wk[:].rearrange("(n p) c -> n p c", p=P)
    wv3 = wv[:].rearrange("(n p) c -> n p c", p=P)
    wo3 = wo[:].rearrange("(n p) d -> n p d", p=P)
    bq3 = bq[:].rearrange("(n p) o -> n p o", p=P)
    bk3 = bk[:].rearrange("(n p) o -> n p o", p=P)
    od3 = od[:].rearrange("(n p) s -> n p s", p=P)
    odx3 = odx[:].rearrange("(n p) s -> n p s", p=P)

    with tile.TileContext(nc) as tc:
        with (
            tc.tile_pool(name="wpool", bufs=1) as wpool,
            tc.tile_pool(name="qkpool", bufs=1) as qkpool,
            tc.tile_pool(name="vpool", bufs=1) as vpool,
            tc.tile_pool(name="maskpool", bufs=1) as maskpool,
            tc.tile_pool(name="cpool", bufs=1) as cpool,
            tc.tile_pool(name="xpool", bufs=1) as xpool,
            tc.tile_pool(name="pmpool", bufs=1) as pmpool,
            tc.tile_pool(name="npool", bufs=1) as npool,
            tc.tile_pool(name="opool", bufs=1) as opool,
        ):
            # ---------------- DMA issue (t=0), one queue per engine -------
            # SP: weights/biases, then xv (reusing xk's sbuf slots), later od.
            # ACT: xk only (completes before the first exp).
            # DVE: xq only (completes before the first mask-mul).
            # Pool: the 16 mask tiles.
            # each weight tensor is ONE strided DMA into a [128, DK*COLS]
            # tile (8 small DMAs would pay ~0.8us of HWDGE overhead each)
            wk3b = wk[:].rearrange("(n p) c -> p n c", p=P)
            wq3b = wq[:].rearrange("(n p) c -> p n c", p=P)
            wv3b = wv[:].rearrange("(n p) c -> p n c", p=P)
            wo3b = wo[:].rearrange("(n p) d -> p n d", p=P)

            wk_all = wpool.tile([P, DK, COLS], BF16, name="wk_all")
            nc.sync.dma_start(out=wk_all[:], in_=wk3b)
            wk_sb = [wk_all[:, i, :] for i in range(DK)]
            b_sb = {}
            for nm, src in (("bk", bk3), ("bq", bq3)):
                for i in range(COLS // P):
                    t = wpool.tile([P, 1], F32, name=f"{nm}_sb{i}")
                    nc.sync.dma_start(out=t[:], in_=src[i])
                    b_sb[(nm, i)] = t
            wq_all = wpool.tile([P, DK, COLS], BF16, name="wq_all")
            nc.sync.dma_start(out=wq_all[:], in_=wq3b)
            wq_sb = [wq_all[:, i, :] for i in range(DK)]

            # x streams split across queues so neither gates a projection:
            # ACT takes xk0-3 + xq0-3 (done well before the first exp), SP
            # takes xk4-7 + xq4-7 between the weight loads, Pool takes xv
            # (reusing the xk sbuf slots once K-proj drains them) and then
            # the 16 mask tiles.
            xk_t, xq_t, xv_t = [], [], []
            for i in range(DK):
                t = xpool.tile([P, S], BF16, name="xk", tag=f"xk{i}", bufs=1)
                # xk0 gates the very first matmul: the Pool/SWDGE queue
                # delivers it ~0.9us sooner than ACT's HWDGE path
                eng = nc.gpsimd if i == 0 else (nc.scalar if i < 4 else nc.sync)
                eng.dma_start(out=t[:], in_=xkT3[i])
                xk_t.append(t)
            for i in range(DK):
                t = xpool.tile([P, S], BF16, name="xq", tag=f"xq{i}", bufs=1)
                (nc.scalar if i < 4 else nc.sync).dma_start(out=t[:], in_=xqT3[i])
                xq_t.append(t)
            for i in range(DK):
                t = xpool.tile([P, S], BF16, name="xv", tag=f"xv{i}", bufs=1)
                nc.gpsimd.dma_start(out=t[:], in_=xvT3[i])
                xv_t.append(t)

            # mask tiles are resident only for the CURRENT sq block
            # ([128, SQB] halves, one generation per block) — this halves
            # their sbuf footprint so xv gets its own slots above.  The
            # sqb1 generation is DMA'd during sqb0's last head.
            def mask_dma(skc, sqb):
                t = maskpool.tile(
                    [P, SQB], BF16, name=f"m{skc}", tag=f"m{skc}", bufs=1
                )
                nc.gpsimd.dma_start(
                    out=t[:], in_=mT3[skc][:, sqb * SQB : (sqb + 1) * SQB]
                )
                return t

            m_sb = [mask_dma(i, 0) for i in range(SKT)]

            # wv/wo follow the x streams on SP (needed at ~31us / ~85us)
            wv_all = wpool.tile([P, DK, COLS], BF16, name="wv_all")
            nc.sync.dma_start(out=wv_all[:], in_=wv3b)
            wv_sb = [wv_all[:, i, :] for i in range(DK)]
            wo_all = wpool.tile([P, COLS // P, D], BF16, name="wo_all")
            nc.sync.dma_start(out=wo_all[:], in_=wo3b)
            wo_sb = [wo_all[:, i, :] for i in range(COLS // P)]

            # Preload the ACT exp table (~1.3us) while ACT is otherwise idle
            # (right after its xk DMAs), so the first exp doesn't pay it.
            warm = npool.tile([P, 1], F32, name="warm", tag="warm", bufs=1)
            nc.scalar.activation(
                warm[:], b_sb[("bk", 0)][:], mybir.ActivationFunctionType.Exp
            )

            # concat^T (normalized attention outputs, head-major columns)
            concat_sb = [
                cpool.tile([P, S], BF16, name=f"concat_sb{i}")
                for i in range(COLS // P)
            ]

            # ---------------- K then Q projections ----------------------
            # qhT[c, s] = (x @ W + b)^T: lhsT = W chunk [128d, 128c]
            # (stationary), rhs = xT chunk [128d, 512s] -> psum [128c, 512s],
            # accumulated over the 8 d-chunks.  K first (its DMA stream and
            # weights land first), then Q reusing the same psum banks.
            # Only the c0 halves of the K/Q projections (heads 0/1) run
            # before attention; the c1 halves are deferred [128,512]-chunk
            # pieces drained into h0/h1's PE slack (they are needed first
            # by h2, ~35us later).  PE is in-order, so this pulls the first
            # scores ~12us earlier.  Evac+bias runs in [128,512] quarters
            # so the psum bank's next user starts a quarter-latency later.
            qhT_sb = {}
            qhT_sb[("k", 1)] = qkpool.tile([P, S], BF16, name="khT1")
            qhT_sb[("q", 1)] = qkpool.tile([P, S], BF16, name="qhT1")
            with tc.tile_pool(name="ps_proj", bufs=1, space="PSUM") as psp:
                def early_c1_piece(tname, x_t, w_sb, sb):
                    pst = psp.tile([P, 512], F32, name="pc1e", tag="pc1e", bufs=2)
                    for dk in range(DK):
                        nc.tensor.matmul(
                            pst[:],
                            lhsT=w_sb[dk][:, P:COLS],
                            rhs=x_t[dk][:, sb * 512 : (sb + 1) * 512],
                            start=(dk == 0),
                            stop=(dk == DK - 1),
                        )
                    # evac on ACT (idle between its DMA stream and the
                    # first exp), keeping DVE clear for Q-c0's bias which
                    # gates the first scores
                    nc.scalar.add(
                        qhT_sb[(tname, 1)][:, sb * 512 : (sb + 1) * 512],
                        pst[:],
                        b_sb[(f"b{tname}", 1)][:],
                    )

                for tname, x_t, w_sb in (("k", xk_t, wk_sb), ("q", xq_t, wq_sb)):
                    pst = psp.tile([P, S], F32, name=f"psp_{tname}0", tag="psp0")
                    for dk in range(DK):
                        for sb in range(S // 512):
                            nc.tensor.matmul(
                                pst[:, sb * 512 : (sb + 1) * 512],
                                lhsT=w_sb[dk][:, 0:P],
                                rhs=x_t[dk][:, sb * 512 : (sb + 1) * 512],
                                start=(dk == 0),
                                stop=(dk == DK - 1),
                            )
                        # Q-c0's dk matmuls are gated by the xq DMA stream;
                        # fill the PE stalls with K-c1 chunk pieces (their
                        # xk inputs are already resident)
                        if tname == "q" and 1 <= dk <= 4:
                            early_c1_piece("k", xk_t, wk_sb, dk - 1)
                    t = qkpool.tile([P, S], BF16, name=f"{tname}hT0")
                    for qtr in range(4):
                        hs = slice(qtr * 512, (qtr + 1) * 512)
                        nc.vector.tensor_scalar_add(
                            t[:, hs], pst[:, hs], b_sb[(f"b{tname}", 0)][:]
                        )
                    qhT_sb[(tname, 0)] = t

            # ---------------- attention ---------------------------------
            # sqb-outer / head-inner.  V-projection pieces are interleaved
            # into (sqb0, h0); O-projection of sqb_i is interleaved into
            # sqb_{i+1}'s attention; the last sqb's O-proj is split by
            # head-pair with the cc1 half as the (pipelined) tail.
            vh_sb = [None] * SKT

            with (
                tc.tile_pool(name="ps_s", bufs=1, space="PSUM") as ps_s_pool,
                tc.tile_pool(name="ps_pv", bufs=1, space="PSUM") as ps_pv_pool,
            ):
                def emit_vproj(skt):
                    # one V-proj piece: vh[skt] = (xv @ Wv)[skt block] + ones
                    psv = ps_pv_pool.tile(
                        [P, 512], F32, name="psv", tag="scratch", bufs=2
                    )
                    for dk in range(DK):
                        nc.tensor.matmul(
                            psv[:, 0:COLS],
                            lhsT=xv_t[dk][:, skt * P : (skt + 1) * P],
                            rhs=wv_sb[dk][:],
                            start=(dk == 0),
                            stop=(dk == DK - 1),
                        )
                    vt = vpool.tile([P, NH, HD + 1], BF16, name=f"vh_sb{skt}")
                    nc.scalar.copy(
                        vt[:, :, 0:HD],
                        psv[:, 0:COLS].rearrange("p (h d) -> p h d", h=NH),
                    )
                    nc.gpsimd.memset(vt[:, :, HD], 1.0)
                    vh_sb[skt] = vt

                def emit_oproj_piece(sqb, dc, half, ccs, dest3, dq0, evac="pool", dma=None, pool=None):
                    # po[128d, 512sq] = sum_cc wo_cc^T @ concat_cc
                    q0 = sqb * SQB + half * 512
                    if pool == "pss":
                        # tail only: the attention's score psum banks are
                        # dead, reuse them as extra O-proj slots
                        po_w = ps_s_pool.tile([P, SQB], F32, name="pss", tag="pss", bufs=2)
                        po = po_w[:, 0:512]
                    else:
                        po = ps_pv_pool.tile(
                            [P, 512], F32, name="po", tag="scratch", bufs=2
                        )[:]
                    for j, cc in enumerate(ccs):
                        nc.tensor.matmul(
                            po[:],
                            lhsT=wo_sb[cc][:, dc * P : (dc + 1) * P],
                            rhs=concat_sb[cc][:, q0 : q0 + 512],
                            start=(j == 0),
                            stop=(j == len(ccs) - 1),
                        )
                    oev = opool.tile([P, 512], BF16, name="oev", tag="oev", bufs=6)
                    if evac == "act":
                        nc.scalar.copy(oev[:], po)
                    else:
                        nc.vector.tensor_copy(oev[:], po)
                    (dma or nc.sync).dma_start(
                        out=dest3[dc][:, dq0 + half * 512 : dq0 + half * 512 + 512],
                        in_=oev[:],
                    )

                ones64 = npool.tile([HD + 1, HD], F32, name="ones64", tag="ones64", bufs=1)
                nc.vector.memset(ones64[:], 1.0)

                # deferred PE pieces, drained one per skc iteration.  An
                # entry flagged needs_concat (O-proj) may only run once the
                # previous head's concat writes are all emitted.
                pending = []

                def drain(n):
                    k = 0
                    while pending and k < n:
                        fn, needs_concat = pending[0]
                        if needs_concat and carry_norm:
                            break
                        pending.pop(0)
                        fn()
                        k += 1

                def emit_proj_piece(tname, x_t, w_sb, sb):
                    # c1 half of the K/Q projection, one [128,512] chunk
                    pst = ps_pv_pool.tile(
                        [P, 512], F32, name="pc1", tag="scratch", bufs=2
                    )
                    for dk in range(DK):
                        nc.tensor.matmul(
                            pst[:],
                            lhsT=w_sb[dk][:, P:COLS],
                            rhs=x_t[dk][:, sb * 512 : (sb + 1) * 512],
                            start=(dk == 0),
                            stop=(dk == DK - 1),
                        )
                    # evac on ACT (idle between its DMA stream and the
                    # first exp), keeping DVE clear for Q-c0's bias which
                    # gates the first scores
                    nc.scalar.add(
                        qhT_sb[(tname, 1)][:, sb * 512 : (sb + 1) * 512],
                        pst[:],
                        b_sb[(f"b{tname}", 1)][:],
                    )

                # K-c1 ran early (interleaved into Q-c0); Q-c1 chunks are
                # drained into h0/h1's slack (h2 needs chunks 0/1 first)
                for sb in range(4):
                    pending.append(
                        (
                            lambda sb=sb: emit_proj_piece("q", xq_t, wq_sb, sb),
                            False,
                        )
                    )

                def make_norm(pv, h, ht, q0, fast=False):
                    # normalization for head h's pv accumulator, cut into 5
                    # pieces spread over the next head's early iterations:
                    # [dnc evac both halves] [recip0+bcast0] [mul0]
                    # [recip1+bcast1] [mul1].  Evac halves go to Pool+DVE in
                    # parallel so the psum slot frees fast.  The denominator
                    # reciprocal runs in-lane on partition 64 and is
                    # broadcast to partitions 0-63 by a K=1 f32r matmul into
                    # a scratch psum bank.  Odd heads land in concat rows
                    # 64:128 via a gpsimd DMA hop (DVE is lane-locked).
                    dnc = npool.tile(
                        [HD + 1, SQB], F32, name="dnc", tag="dnc", bufs=2
                    )

                    def evac_half(i):
                        def fn():
                            nc.vector.tensor_copy(
                                dnc[:, i * 512 : (i + 1) * 512], pv[i][:]
                            )
                        return fn

                    def evac():
                        evac_half(0)()
                        evac_half(1)()

                    rbs = {}

                    def recip_piece(half):
                        def fn():
                            sl = slice(half * 512, half * 512 + 512)
                            if fast:
                                # tail-critical: in-lane reciprocal on
                                # partition 64 + fp32 K=1 broadcast matmul
                                # (shorter chain than the hop+broadcast)
                                nc.vector.reciprocal(
                                    out=dnc[HD : HD + 1, sl], in_=dnc[HD : HD + 1, sl]
                                )
                                rb = ps_pv_pool.tile(
                                    [P, 512], F32, name="rbf", tag="scratch", bufs=2
                                )
                                nc.tensor.matmul(
                                    rb[0:HD, :],
                                    lhsT=ones64[HD : HD + 1, :],
                                    rhs=dnc[HD : HD + 1, sl],
                                    start=True,
                                    stop=True,
                                )
                                rbs[half] = rb[0:HD, :]
                                return
                            # hop the denominator row to partition 0 (DVE is
                            # lane-locked; reciprocal_approx_fast and
                            # partition_broadcast are partition-0 ops)
                            dn0 = npool.tile([1, 512], F32, name="dn0", tag="dn0", bufs=1)
                            nc.gpsimd.dma_start(out=dn0[:], in_=dnc[HD : HD + 1, sl])
                            nc.vector.reciprocal_approx_fast(out=dn0[:], in_=dn0[:])
                            rb = npool.tile(
                                [HD, 512], F32, name=f"rb{half}", tag=f"rb{half}", bufs=1
                            )
                            nc.gpsimd.partition_broadcast(rb[:], dn0[:])
                            rbs[half] = rb[:]
                        return fn

                    def mul_piece(half):
                        def fn():
                            sl = slice(half * 512, half * 512 + 512)
                            cs = slice(q0 + half * 512, q0 + half * 512 + 512)
                            if h % 2 == 0:
                                nc.vector.tensor_mul(
                                    concat_sb[ht][0:HD, cs], dnc[0:HD, sl], rbs[half]
                                )
                            else:
                                tmp = npool.tile([HD, 512], BF16, name="tmpn", tag="tmpn", bufs=1)
                                nc.vector.tensor_mul(tmp[:], dnc[0:HD, sl], rbs[half])
                                nc.gpsimd.dma_start(out=concat_sb[ht][HD:P, cs], in_=tmp[:])
                        return fn

                    if fast:
                        # tail: per-half chains so the first O-proj pieces
                        # start after half0's mul instead of the full chain
                        return [
                            evac_half(0), recip_piece(0), mul_piece(0),
                            evac_half(1), recip_piece(1), mul_piece(1),
                        ]
                    return [evac, recip_piece(0), mul_piece(0), recip_piece(1), mul_piece(1)]

                # carried finishers from the previous head:
                # [PV(prev,14), PV(prev,15)] then the 5 norm pieces
                carry_pv = []
                carry_norm = []

                m_cur = m_sb
                m_next = [None] * SKT
                for sqb in range(NSQB):
                    q0 = sqb * SQB
                    last_sqb = sqb == NSQB - 1
                    if sqb == 1:
                        m_cur = m_next
                    heads = [0, 1, 3, 2] if last_sqb else [0, 1, 2, 3]
                    for hi, h in enumerate(heads):
                        ht, hp = h // 2, HD * (h % 2)
                        qT = qhT_sb[("q", ht)]
                        kT = qhT_sb[("k", ht)]
                        pv = [
                            ps_pv_pool.tile(
                                [HD + 1, 512], F32, name=f"pspv{i}", tag=f"pspv{i}", bufs=1
                            )
                            for i in range(2)
                        ]

                        def emit_pv(pm_t, skc, pv=pv, h=h):
                            for i in range(SQB // 512):
                                nc.tensor.matmul(
                                    pv[i][:],
                                    lhsT=vh_sb[skc][:, h, :],
                                    rhs=pm_t[:, i * 512 : (i + 1) * 512],
                                    start=(skc == 0),
                                    stop=(skc == SKT - 1),
                                )

                        pms = [None] * SKT
                        for skc in range(SKT):
                            ss = ps_s_pool.tile(
                                [P, SQB], F32, name="pss", tag="pss", bufs=2
                            )
                            for i in range(SQB // 512):
                                nc.tensor.matmul(
                                    ss[:, i * 512 : (i + 1) * 512],
                                    lhsT=kT[hp : hp + HD, skc * P : (skc + 1) * P],
                                    rhs=qT[hp : hp + HD, q0 + i * 512 : q0 + (i + 1) * 512],
                                    start=True,
                                    stop=True,
                                )
                            # PE-order fillers: carried PV flushes of the
                            # previous head at skc 0/1, then this head's PV
                            # trailing by two chunks; V-proj pieces ride in
                            # (sqb0, h0); O-proj pieces drain once concat of
                            # their block is complete (skc>=6 guard).
                            if skc <= 1 and carry_pv:
                                carry_pv.pop(0)()
                            if sqb == 0 and hi == 0:
                                emit_vproj(skc)
                            elif skc % 2 == 1 or len(pending) >= 17:
                                drain(1)
                            if skc >= 2:
                                emit_pv(pms[skc - 2], skc - 2)
                            pm = pmpool.tile([P, SQB], BF16, name="pm", tag="pm", bufs=4)
                            nc.scalar.activation(
                                pm[:], ss[:], mybir.ActivationFunctionType.Exp
                            )
                            # every 4th mask-mul rides on the idle gpsimd
                            # engine to relieve DVE pacing (pm is consumed
                            # two iterations later, so the slower engine's
                            # latency is hidden)
                            meng = nc.gpsimd if skc % 4 == 3 else nc.vector
                            meng.tensor_mul(pm[:], pm[:], m_cur[skc][:])
                            pms[skc] = pm
                            # refresh this mask slot with the next block's
                            # half once its last reader is emitted
                            if sqb == 0 and hi == NH - 1 and NSQB > 1:
                                m_next[skc] = mask_dma(skc, 1)
                            # previous head's norm: evac right after the PV
                            # flushes (frees its psum slot), remaining
                            # pieces one per iteration
                            if skc >= 1 and carry_norm:
                                carry_norm.pop(0)()
                        # head end: set up finishers for this head
                        assert not carry_pv and not carry_norm
                        carry_pv = [
                            lambda skc=skc_, f=emit_pv, pms=pms: f(pms[skc], skc)
                            for skc_ in (SKT - 2, SKT - 1)
                        ]
                        carry_norm = make_norm(
                            pv, h, ht, q0, fast=last_sqb and hi == NH - 1
                        )

                        if last_sqb and hi == 1:
                            # cc0 half of the last block's O-proj runs
                            # during the last two heads' attention
                            for dc in range(D // P):
                                for half in range(2):
                                    pending.append(
                                        (
                                            lambda dc=dc, half=half, sqb=sqb: emit_oproj_piece(
                                                sqb, dc, half, [0], od3, sqb * SQB
                                            ),
                                            True,
                                        )
                                    )
                    # end heads
                    if not last_sqb:
                        # whole-block O-proj pieces drained in the next block
                        for dc in range(D // P):
                            for half in range(2):
                                pending.append(
                                    (
                                        lambda sqb=sqb, dc=dc, half=half: emit_oproj_piece(
                                            sqb, dc, half, [0, 1], od3, sqb * SQB
                                        ),
                                        True,
                                    )
                                )
                # final head: flush PVs and run its norm immediately, then
                # the cc1 tail pieces (half-outer so half0 starts after the
                # first mul), written to the odx partial
                for f in carry_pv:
                    f()
                carry_pv = []
                for fn, _ in pending:
                    fn()
                pending = []
                ev0, r0, m0, ev1, r1, m1 = carry_norm
                ev0(); r0(); m0(); ev1(); r1()
                for dc in range(D // P):
                    if dc == 2:
                        m1()
                    emit_oproj_piece(
                        NSQB - 1, dc, 0, [1], odx3, 0,
                        evac="dve" if dc % 2 else "act",
                        pool="pss" if dc % 2 else None,
                        dma=nc.gpsimd if dc % 2 == 1 else nc.sync,
                    )
                for dc in range(D // P):
                    emit_oproj_piece(
                        NSQB - 1, dc, 1, [1], odx3, 0,
                        evac="dve" if dc % 2 else "act",
                        pool="pss" if dc % 2 else None,
                        dma=nc.gpsimd if dc % 2 == 1 else nc.sync,
                    )

    nc.compile()
    return nc


def get_program():
    if "nc" not in _CACHE:
        _CACHE["nc"] = _build_program()
    return _CACHE["nc"]


def make_in_maps(q, k, v, mask, Wq, bq, Wk, bk, Wv, bv, Wo, bo, gate):
    """Host-side sharding: per-core input dict (all numpy)."""
    q, k, v = (np.asarray(a, np.float32) for a in (q, k, v))
    mask = np.asarray(mask)
    Wq, bq, Wk, bk, Wv, bv, Wo, bo, gate = (
        np.asarray(a, np.float32) for a in (Wq, bq, Wk, bk, Wv, bv, Wo, bo, gate)
    )
    scale = 1.0 / np.sqrt(HD)
    xT = {}
    for b in range(B):
        xT[("q", b)] = np.ascontiguousarray(q[b].T).astype(NPBF16)
        xT[("k", b)] = np.ascontiguousarray(k[b].T).astype(NPBF16)
        xT[("v", b)] = np.ascontiguousarray(v[b].T).astype(NPBF16)
        xT[("m", b)] = np.ascontiguousarray(mask[b].T).astype(NPBF16)

    in_maps = []
    for c in range(NCORES):
        b = c // (NCORES // B)
        g = c % (NCORES // B)
        cols = slice(g * COLS, (g + 1) * COLS)
        gate_cols = np.repeat(gate[g * NH : (g + 1) * NH], HD)  # [256]
        in_maps.append(
            {
                "xqT": xT[("q", b)],
                "xkT": xT[("k", b)],
                "xvT": xT[("v", b)],
                "mT": xT[("m", b)],
                # fold the 1/sqrt(hd) score scale into Wq and bq;
                # fold the per-head gate into Wv (bv handled on host)
                "wq": (Wq[:, cols] * scale).astype(NPBF16),
                "wk": Wk[:, cols].astype(NPBF16),
                "wv": (Wv[:, cols] * gate_cols[None, :]).astype(NPBF16),
                "wo": np.ascontiguousarray(Wo[cols, :]).astype(NPBF16),
                "bq": (bq[cols] * scale).astype(np.float32).reshape(COLS, 1),
                "bk": bk[cols].astype(np.float32).reshape(COLS, 1),
            }
        )
    return in_maps


LAST_RESULTS = None


def kernel(q, k, v, mask, Wq, bq, Wk, bk, Wv, bv, Wo, bo, gate, trace=False):
    global LAST_RESULTS
    nc = get_program()
    in_maps = make_in_maps(q, k, v, mask, Wq, bq, Wk, bk, Wv, bv, Wo, bo, gate)
    res = run_bass_kernel_spmd(nc, in_maps, core_ids=list(range(NCORES)), trace=trace)
    LAST_RESULTS = res

    bv_ = np.asarray(bv, np.float32)
    bo_ = np.asarray(bo, np.float32)
    gate_ = np.asarray(gate, np.float32)
    Wo_ = np.asarray(Wo, np.float32)
    # attention rows sum to 1, so the bv term is a constant vector:
    # concat-level constant = repeat(gate, hd) * bv, projected through Wo.
    const = (np.repeat(gate_, HD) * bv_) @ Wo_ + bo_

    out = np.zeros((B, S, D), np.float32)
    for c in range(NCORES):
        b = c // (NCORES // B)
        out[b] += res.results[c]["od"].astype(np.float32).T
        # the last sq block's cc1 contribution is a separate partial
        out[b, (NSQB - 1) * SQB :, :] += res.results[c]["odx"].astype(np.float32).T
    out += const[None, None, :]
    return out


# revision 94
# speedup vs baseline: 1.1472x; 1.0021x over previous
"""Multi-head attention on 8 Trainium2 NeuronCores.

Problem: B=2, S=2048, D=1024, H=16 heads (head_dim 64), boolean mask,
per-head gate, QKV/out linear projections.

Sharding: core c handles batch b=c//4 and heads 4*(c%4)..4*(c%4)+3.
Each core computes its 4 heads' attention and the partial output
projection (contribution of its 256 concat columns through Wo); the host
sums the 4 partials per batch and adds the constant terms (bo, and the
bv/gate contribution which is constant because attention rows sum to 1).

Schedule (PE-bound design, tuned against the CoreSim cost model; all
constructs are neuronxcc/BIR-legal — gpsimd never touches PSUM, no f32r):
  - ACT (scalar) engine runs the 128 x [128,1024] exp stream plus only
    the xk/xq DMAs that complete before the first exp.  Other DMAs live
    on the SP and Pool queues; each weight tensor is one strided DMA.
  - PE is in-order, so emission order IS the schedule.  Only the c0
    halves of the K/Q projections run before attention (first scores at
    ~20us); the c1 halves run as deferred [128,512] pieces inside the
    h0/h1 PE slack (K-c1 interleaves into Q-c0's DMA-gated stalls).
  - scores computed transposed [sk, sq]; exp is a pure ACT pass, mask is
    a multiplicative fp16 tensor_mul on DVE, softmax denominator rides as
    a 65th "ones" column of the PV stationary.  PV trails scores by two
    chunks and the last two PVs + the (5-piece, 512-halved) normalization
    of each head spread across the next head's iterations, so the psum
    slots hand over without stalling PE.
  - V projection is single-pass (skt-outer, xv resident) into 1-bank
    psum scratch slots during h0; evacuation on ACT (slack there).
  - O projection is cut into [128,512] 1-bank pieces drained one per two
    iterations: sqb0's pieces into sqb1's h0/h1, the last block's cc0
    pieces into its h3/h2, and its cc1 pieces form a pipelined tail
    written to the odx partial (summed on host).  Mask tiles are resident
    per-sq-block only (halved footprint funds xv's own sbuf slots).
  - od/odx partials are written fp16 (host accumulates in fp32).
"""

import sys

if "/opt/trn_rl_repo" not in sys.path:
    sys.path.insert(0, "/opt/trn_rl_repo")

import numpy as np

import concourse.bass as bass
import concourse.bacc as bacc
import concourse.mybir as mybir
import concourse.tile as tile
from concourse.bass_utils import run_bass_kernel_spmd

BF16 = mybir.dt.float16  # fp16: same speed as bf16, 3 more mantissa bits
F32 = mybir.dt.float32
NPBF16 = np.float16

P = 128
B, S, D = 2, 2048, 1024
HEADS, HD = 16, 64
NCORES = 8
NH = HEADS // (NCORES // B)  # heads per core = 4
COLS = NH * HD               # 256 concat columns per core
DK = D // P                  # 8 contraction chunks for the projections
SKT = S // P                 # 16 key chunks
SQB = 1024                   # query block width in the attention loop
NSQB = S // SQB

_CACHE = {}


def _build_program():
    nc = bacc.Bacc("TRN2", debug=False)

    xqT = nc.declare_dram_parameter("xqT", [D, S], BF16, isOutput=False)
    xkT = nc.declare_dram_parameter("xkT", [D, S], BF16, isOutput=False)
    xvT = nc.declare_dram_parameter("xvT", [D, S], BF16, isOutput=False)
    mT = nc.declare_dram_parameter("mT", [S, S], BF16, isOutput=False)
    wq = nc.declare_dram_parameter("wq", [D, COLS], BF16, isOutput=False)
    wk = nc.declare_dram_parameter("wk", [D, COLS], BF16, isOutput=False)
    wv = nc.declare_dram_parameter("wv", [D, COLS], BF16, isOutput=False)
    wo = nc.declare_dram_parameter("wo", [COLS, D], BF16, isOutput=False)
    bq = nc.declare_dram_parameter("bq", [COLS, 1], F32, isOutput=False)
    bk = nc.declare_dram_parameter("bk", [COLS, 1], F32, isOutput=False)
    od = nc.declare_dram_parameter("od", [D, S], BF16, isOutput=True)
    # cc1 (heads 2,3) partial of the LAST sq block, summed on host
    odx = nc.declare_dram_parameter("odx", [D, SQB], BF16, isOutput=True)

    xqT3 = xqT[:].rearrange("(n p) s -> n p s", p=P)
    xkT3 = xkT[:].rearrange("(n p) s -> n p s", p=P)
    xvT3 = xvT[:].rearrange("(n p) s -> n p s", p=P)
    mT3 = mT[:].rearrange("(n p) s -> n p s", p=P)
    wq3 = wq[:].rearrange("(n p) c -> n p c", p=P)
    wk3 = wk[:].rearrange("(n p) c -> n p c", p=P)
    wv3 = wv[:].rearrange("(n p) c -> n p c", p=P)
    wo3 = wo[:].rearrange("(n p) d -> n p d", p=P)
    bq3 = bq[:].rearrange("(n p) o -> n p o", p=P)
    bk3 = bk[:].rearrange("(n p) o -> n p o", p=P)
    od3 = od[:].rearrange("(n p) s -> n p s", p=P)
    odx3 = odx[:].rearrange("(n p) s -> n p s", p=P)

    with tile.TileContext(nc) as tc:
        with (
            tc.tile_pool(name="wpool", bufs=1) as wpool,
            tc.tile_pool(name="qkpool", bufs=1) as qkpool,
            tc.tile_pool(name="vpool", bufs=1) as vpool,
            tc.tile_pool(name="maskpool", bufs=1) as maskpool,
            tc.tile_pool(name="cpool", bufs=1) as cpool,
            tc.tile_pool(name="xpool", bufs=1) as xpool,
            tc.tile_pool(name="pmpool", bufs=1) as pmpool,
            tc.tile_pool(name="npool", bufs=1) as npool,
            tc.tile_pool(name="opool", bufs=1) as opool,
        ):
            # ---------------- DMA issue (t=0), one queue per engine -------
            # SP: weights/biases, then xv (reusing xk's sbuf slots), later od.
            # ACT: xk only (completes before the first exp).
            # DVE: xq only (completes before the first mask-mul).
            # Pool: the 16 mask tiles.
            # each weight tensor is ONE strided DMA into a [128, DK*COLS]
            # tile (8 small DMAs would pay ~0.8us of HWDGE overhead each)
            wk3b = wk[:].rearrange("(n p) c -> p n c", p=P)
            wq3b = wq[:].rearrange("(n p) c -> p n c", p=P)
            wv3b = wv[:].rearrange("(n p) c -> p n c", p=P)
            wo3b = wo[:].rearrange("(n p) d -> p n d", p=P)

            wk_all = wpool.tile([P, DK, COLS], BF16, name="wk_all")
            nc.sync.dma_start(out=wk_all[:], in_=wk3b)
            wk_sb = [wk_all[:, i, :] for i in range(DK)]
            b_sb = {}
            for nm, src in (("bk", bk3), ("bq", bq3)):
                for i in range(COLS // P):
                    t = wpool.tile([P, 1], F32, name=f"{nm}_sb{i}")
                    nc.sync.dma_start(out=t[:], in_=src[i])
                    b_sb[(nm, i)] = t
            wq_all = wpool.tile([P, DK, COLS], BF16, name="wq_all")
            nc.sync.dma_start(out=wq_all[:], in_=wq3b)
            wq_sb = [wq_all[:, i, :] for i in range(DK)]

            # x streams split across queues so neither gates a projection:
            # ACT takes xk0-3 + xq0-3 (done well before the first exp), SP
            # takes xk4-7 + xq4-7 between the weight loads, Pool takes xv
            # (reusing the xk sbuf slots once K-proj drains them) and then
            # the 16 mask tiles.
            xk_t, xq_t, xv_t = [], [], []
            for i in range(DK):
                t = xpool.tile([P, S], BF16, name="xk", tag=f"xk{i}", bufs=1)
                # xk0 gates the very first matmul: the Pool/SWDGE queue
                # delivers it ~0.9us sooner than ACT's HWDGE path
                eng = nc.gpsimd if i == 0 else (nc.scalar if i < 4 else nc.sync)
                eng.dma_start(out=t[:], in_=xkT3[i])
                xk_t.append(t)
            for i in range(DK):
                t = xpool.tile([P, S], BF16, name="xq", tag=f"xq{i}", bufs=1)
                (nc.scalar if i < 4 else nc.sync).dma_start(out=t[:], in_=xqT3[i])
                xq_t.append(t)
            for i in range(DK):
                t = xpool.tile([P, S], BF16, name="xv", tag=f"xv{i}", bufs=1)
                nc.gpsimd.dma_start(out=t[:], in_=xvT3[i])
                xv_t.append(t)

            # mask tiles are resident only for the CURRENT sq block
            # ([128, SQB] halves, one generation per block) — this halves
            # their sbuf footprint so xv gets its own slots above.  The
            # sqb1 generation is DMA'd during sqb0's last head.
            def mask_dma(skc, sqb):
                t = maskpool.tile(
                    [P, SQB], BF16, name=f"m{skc}", tag=f"m{skc}", bufs=1
                )
                nc.gpsimd.dma_start(
                    out=t[:], in_=mT3[skc][:, sqb * SQB : (sqb + 1) * SQB]
                )
                return t

            m_sb = [mask_dma(i, 0) for i in range(SKT)]

            # wv/wo follow the x streams on SP (needed at ~31us / ~85us)
            wv_all = wpool.tile([P, DK, COLS], BF16, name="wv_all")
            nc.sync.dma_start(out=wv_all[:], in_=wv3b)
            wv_sb = [wv_all[:, i, :] for i in range(DK)]
            wo_all = wpool.tile([P, COLS // P, D], BF16, name="wo_all")
            nc.sync.dma_start(out=wo_all[:], in_=wo3b)
            wo_sb = [wo_all[:, i, :] for i in range(COLS // P)]

            # Preload the ACT exp table (~1.3us) while ACT is otherwise idle
            # (right after its xk DMAs), so the first exp doesn't pay it.
            warm = npool.tile([P, 1], F32, name="warm", tag="warm", bufs=1)
            nc.scalar.activation(
                warm[:], b_sb[("bk", 0)][:], mybir.ActivationFunctionType.Exp
            )

            # concat^T (normalized attention outputs, head-major columns)
            concat_sb = [
                cpool.tile([P, S], BF16, name=f"concat_sb{i}")
                for i in range(COLS // P)
            ]

            # ---------------- K then Q projections ----------------------
            # qhT[c, s] = (x @ W + b)^T: lhsT = W chunk [128d, 128c]
            # (stationary), rhs = xT chunk [128d, 512s] -> psum [128c, 512s],
            # accumulated over the 8 d-chunks.  K first (its DMA stream and
            # weights land first), then Q reusing the same psum banks.
            # Only the c0 halves of the K/Q projections (heads 0/1) run
            # before attention; the c1 halves are deferred [128,512]-chunk
            # pieces drained into h0/h1's PE slack (they are needed first
            # by h2, ~35us later).  PE is in-order, so this pulls the first
            # scores ~12us earlier.  Evac+bias runs in [128,512] quarters
            # so the psum bank's next user starts a quarter-latency later.
            qhT_sb = {}
            qhT_sb[("k", 1)] = qkpool.tile([P, S], BF16, name="khT1")
            qhT_sb[("q", 1)] = qkpool.tile([P, S], BF16, name="qhT1")
            with tc.tile_pool(name="ps_proj", bufs=1, space="PSUM") as psp:
                def early_c1_piece(tname, x_t, w_sb, sb):
                    pst = psp.tile([P, 512], F32, name="pc1e", tag="pc1e", bufs=2)
                    for dk in range(DK):
                        nc.tensor.matmul(
                            pst[:],
                            lhsT=w_sb[dk][:, P:COLS],
                            rhs=x_t[dk][:, sb * 512 : (sb + 1) * 512],
                            start=(dk == 0),
                            stop=(dk == DK - 1),
                        )
                    # evac on ACT (idle between its DMA stream and the
                    # first exp), keeping DVE clear for Q-c0's bias which
                    # gates the first scores
                    nc.scalar.add(
                        qhT_sb[(tname, 1)][:, sb * 512 : (sb + 1) * 512],
                        pst[:],
                        b_sb[(f"b{tname}", 1)][:],
                    )

                for tname, x_t, w_sb in (("k", xk_t, wk_sb), ("q", xq_t, wq_sb)):
                    pst = psp.tile([P, S], F32, name=f"psp_{tname}0", tag="psp0")
                    for dk in range(DK):
                        for sb in range(S // 512):
                            nc.tensor.matmul(
                                pst[:, sb * 512 : (sb + 1) * 512],
                                lhsT=w_sb[dk][:, 0:P],
                                rhs=x_t[dk][:, sb * 512 : (sb + 1) * 512],
                                start=(dk == 0),
                                stop=(dk == DK - 1),
                            )
                        # Q-c0's dk matmuls are gated by the xq DMA stream;
                        # fill the PE stalls with K-c1 chunk pieces (their
                        # xk inputs are already resident)
                        if tname == "q" and 1 <= dk <= 4:
                            early_c1_piece("k", xk_t, wk_sb, dk - 1)
                    t = qkpool.tile([P, S], BF16, name=f"{tname}hT0")
                    for qtr in range(4):
                        hs = slice(qtr * 512, (qtr + 1) * 512)
                        nc.vector.tensor_scalar_add(
                            t[:, hs], pst[:, hs], b_sb[(f"b{tname}", 0)][:]
                        )
                    qhT_sb[(tname, 0)] = t

            # ---------------- attention ---------------------------------
            # sqb-outer / head-inner.  V-projection pieces are interleaved
            # into (sqb0, h0); O-projection of sqb_i is interleaved into
            # sqb_{i+1}'s attention; the last sqb's O-proj is split by
            # head-pair with the cc1 half as the (pipelined) tail.
            vh_sb = [None] * SKT

            with (
                tc.tile_pool(name="ps_s", bufs=1, space="PSUM") as ps_s_pool,
                tc.tile_pool(name="ps_pv", bufs=1, space="PSUM") as ps_pv_pool,
            ):
                def emit_vproj(skt):
                    # one V-proj piece: vh[skt] = (xv @ Wv)[skt block] + ones
                    psv = ps_pv_pool.tile(
                        [P, 512], F32, name="psv", tag="scratch", bufs=2
                    )
                    for dk in range(DK):
                        nc.tensor.matmul(
                            psv[:, 0:COLS],
                            lhsT=xv_t[dk][:, skt * P : (skt + 1) * P],
                            rhs=wv_sb[dk][:],
                            start=(dk == 0),
                            stop=(dk == DK - 1),
                        )
                    vt = vpool.tile([P, NH, HD + 1], BF16, name=f"vh_sb{skt}")
                    nc.scalar.copy(
                        vt[:, :, 0:HD],
                        psv[:, 0:COLS].rearrange("p (h d) -> p h d", h=NH),
                    )
                    nc.gpsimd.memset(vt[:, :, HD], 1.0)
                    vh_sb[skt] = vt

                def emit_oproj_piece(sqb, dc, half, ccs, dest3, dq0, evac="pool", dma=None, pool=None):
                    # po[128d, 512sq] = sum_cc wo_cc^T @ concat_cc
                    q0 = sqb * SQB + half * 512
                    if pool == "pss":
                        # tail only: the attention's score psum banks are
                        # dead, reuse them as extra O-proj slots
                        po_w = ps_s_pool.tile([P, SQB], F32, name="pss", tag="pss", bufs=2)
                        po = po_w[:, 0:512]
                    else:
                        po = ps_pv_pool.tile(
                            [P, 512], F32, name="po", tag="scratch", bufs=2
                        )[:]
                    for j, cc in enumerate(ccs):
                        nc.tensor.matmul(
                            po[:],
                            lhsT=wo_sb[cc][:, dc * P : (dc + 1) * P],
                            rhs=concat_sb[cc][:, q0 : q0 + 512],
                            start=(j == 0),
                            stop=(j == len(ccs) - 1),
                        )
                    oev = opool.tile([P, 512], BF16, name="oev", tag="oev", bufs=6)
                    if evac == "act":
                        nc.scalar.copy(oev[:], po)
                    else:
                        nc.vector.tensor_copy(oev[:], po)
                    (dma or nc.sync).dma_start(
                        out=dest3[dc][:, dq0 + half * 512 : dq0 + half * 512 + 512],
                        in_=oev[:],
                    )

                ones64 = npool.tile([HD + 1, HD], F32, name="ones64", tag="ones64", bufs=1)
                nc.vector.memset(ones64[:], 1.0)

                # deferred PE pieces, drained one per skc iteration.  An
                # entry flagged needs_concat (O-proj) may only run once the
                # previous head's concat writes are all emitted.
                pending = []

                def drain(n):
                    k = 0
                    while pending and k < n:
                        fn, needs_concat = pending[0]
                        if needs_concat and carry_norm:
                            break
                        pending.pop(0)
                        fn()
                        k += 1

                def emit_proj_piece(tname, x_t, w_sb, sb):
                    # c1 half of the K/Q projection, one [128,512] chunk
                    pst = ps_pv_pool.tile(
                        [P, 512], F32, name="pc1", tag="scratch", bufs=2
                    )
                    for dk in range(DK):
                        nc.tensor.matmul(
                            pst[:],
                            lhsT=w_sb[dk][:, P:COLS],
                            rhs=x_t[dk][:, sb * 512 : (sb + 1) * 512],
                            start=(dk == 0),
                            stop=(dk == DK - 1),
                        )
                    # evac on ACT (idle between its DMA stream and the
                    # first exp), keeping DVE clear for Q-c0's bias which
                    # gates the first scores
                    nc.scalar.add(
                        qhT_sb[(tname, 1)][:, sb * 512 : (sb + 1) * 512],
                        pst[:],
                        b_sb[(f"b{tname}", 1)][:],
                    )

                # K-c1 ran early (interleaved into Q-c0); Q-c1 chunks are
                # drained into h0/h1's slack (h2 needs chunks 0/1 first)
                for sb in range(4):
                    pending.append(
                        (
                            lambda sb=sb: emit_proj_piece("q", xq_t, wq_sb, sb),
                            False,
                        )
                    )

                def make_norm(pv, h, ht, q0, fast=False):
                    # normalization for head h's pv accumulator, cut into 5
                    # pieces spread over the next head's early iterations:
                    # [dnc evac both halves] [recip0+bcast0] [mul0]
                    # [recip1+bcast1] [mul1].  Evac halves go to Pool+DVE in
                    # parallel so the psum slot frees fast.  The denominator
                    # reciprocal runs in-lane on partition 64 and is
                    # broadcast to partitions 0-63 by a K=1 f32r matmul into
                    # a scratch psum bank.  Odd heads land in concat rows
                    # 64:128 via a gpsimd DMA hop (DVE is lane-locked).
                    dnc = npool.tile(
                        [HD + 1, SQB], F32, name="dnc", tag="dnc", bufs=2
                    )

                    def evac_half(i):
                        def fn():
                            nc.vector.tensor_copy(
                                dnc[:, i * 512 : (i + 1) * 512], pv[i][:]
                            )
                        return fn

                    def evac():
                        evac_half(0)()
                        evac_half(1)()

                    rbs = {}

                    def recip_piece(half):
                        def fn():
                            sl = slice(half * 512, half * 512 + 512)
                            if fast:
                                # tail-critical: in-lane reciprocal on
                                # partition 64 + fp32 K=1 broadcast matmul
                                # (shorter chain than the hop+broadcast)
                                nc.vector.reciprocal(
                                    out=dnc[HD : HD + 1, sl], in_=dnc[HD : HD + 1, sl]
                                )
                                rb = ps_pv_pool.tile(
                                    [P, 512], F32, name="rbf", tag="scratch", bufs=2
                                )
                                nc.tensor.matmul(
                                    rb[0:HD, :],
                                    lhsT=ones64[HD : HD + 1, :],
                                    rhs=dnc[HD : HD + 1, sl],
                                    start=True,
                                    stop=True,
                                )
                                rbs[half] = rb[0:HD, :]
                                return
                            # hop the denominator row to partition 0 (DVE is
                            # lane-locked; reciprocal_approx_fast and
                            # partition_broadcast are partition-0 ops)
                            dn0 = npool.tile([1, 512], F32, name="dn0", tag="dn0", bufs=1)
                            nc.gpsimd.dma_start(out=dn0[:], in_=dnc[HD : HD + 1, sl])
                            nc.vector.reciprocal_approx_fast(out=dn0[:], in_=dn0[:])
                            rb = npool.tile(
                                [HD, 512], F32, name=f"rb{half}", tag=f"rb{half}", bufs=1
                            )
                            nc.gpsimd.partition_broadcast(rb[:], dn0[:])
                            rbs[half] = rb[:]
                        return fn

                    def mul_piece(half):
                        def fn():
                            sl = slice(half * 512, half * 512 + 512)
                            cs = slice(q0 + half * 512, q0 + half * 512 + 512)
                            if h % 2 == 0:
                                nc.vector.tensor_mul(
                                    concat_sb[ht][0:HD, cs], dnc[0:HD, sl], rbs[half]
                                )
                            else:
                                tmp = npool.tile([HD, 512], BF16, name="tmpn", tag="tmpn", bufs=1)
                                nc.vector.tensor_mul(tmp[:], dnc[0:HD, sl], rbs[half])
                                nc.gpsimd.dma_start(out=concat_sb[ht][HD:P, cs], in_=tmp[:])
                        return fn

                    if fast:
                        # tail: per-half chains so the first O-proj pieces
                        # start after half0's mul instead of the full chain
                        return [
                            evac_half(0), recip_piece(0), mul_piece(0),
                            evac_half(1), recip_piece(1), mul_piece(1),
                        ]
                    return [evac, recip_piece(0), mul_piece(0), recip_piece(1), mul_piece(1)]

                # carried finishers from the previous head:
                # [PV(prev,14), PV(prev,15)] then the 5 norm pieces
                carry_pv = []
                carry_norm = []

                m_cur = m_sb
                m_next = [None] * SKT
                for sqb in range(NSQB):
                    q0 = sqb * SQB
                    last_sqb = sqb == NSQB - 1
                    if sqb == 1:
                        m_cur = m_next
                    heads = [0, 1, 3, 2] if last_sqb else [0, 1, 2, 3]
                    for hi, h in enumerate(heads):
                        ht, hp = h // 2, HD * (h % 2)
                        qT = qhT_sb[("q", ht)]
                        kT = qhT_sb[("k", ht)]
                        pv = [
                            ps_pv_pool.tile(
                                [HD + 1, 512], F32, name=f"pspv{i}", tag=f"pspv{i}", bufs=1
                            )
                            for i in range(2)
                        ]

                        def emit_pv(pm_t, skc, pv=pv, h=h):
                            for i in range(SQB // 512):
                                nc.tensor.matmul(
                                    pv[i][:],
                                    lhsT=vh_sb[skc][:, h, :],
                                    rhs=pm_t[:, i * 512 : (i + 1) * 512],
                                    start=(skc == 0),
                                    stop=(skc == SKT - 1),
                                )

                        pms = [None] * SKT
                        for skc in range(SKT):
                            ss = ps_s_pool.tile(
                                [P, SQB], F32, name="pss", tag="pss", bufs=2
                            )
                            for i in range(SQB // 512):
                                nc.tensor.matmul(
                                    ss[:, i * 512 : (i + 1) * 512],
                                    lhsT=kT[hp : hp + HD, skc * P : (skc + 1) * P],
                                    rhs=qT[hp : hp + HD, q0 + i * 512 : q0 + (i + 1) * 512],
                                    start=True,
                                    stop=True,
                                )
                            # PE-order fillers: carried PV flushes of the
                            # previous head at skc 0/1, then this head's PV
                            # trailing by two chunks; V-proj pieces ride in
                            # (sqb0, h0); O-proj pieces drain once concat of
                            # their block is complete (skc>=6 guard).
                            if skc <= 1 and carry_pv:
                                carry_pv.pop(0)()
                            if sqb == 0 and hi == 0:
                                emit_vproj(skc)
                            elif skc % 2 == 1 or len(pending) >= 17:
                                drain(1)
                            if skc >= 2:
                                emit_pv(pms[skc - 2], skc - 2)
                            pm = pmpool.tile([P, SQB], BF16, name="pm", tag="pm", bufs=4)
                            if last_sqb and hi == NH - 1 and skc == SKT - 1:
                                # final chunk: exp in halves so the last PV
                                # (and the tail behind it) starts earlier
                                for eh in range(2):
                                    es = slice(eh * 512, eh * 512 + 512)
                                    nc.scalar.activation(
                                        pm[:, es], ss[:, es],
                                        mybir.ActivationFunctionType.Exp,
                                    )
                            else:
                                nc.scalar.activation(
                                    pm[:], ss[:], mybir.ActivationFunctionType.Exp
                                )
                            # every 4th mask-mul rides on the idle gpsimd
                            # engine to relieve DVE pacing (pm is consumed
                            # two iterations later, so the slower engine's
                            # latency is hidden)
                            # higher gpsimd share where DVE also carries
                            # the previous block's O-proj evacuations
                            pool_mul = skc % 4 == 3 or (
                                sqb == 1 and hi <= 1 and skc % 4 == 1
                            )
                            meng = nc.gpsimd if pool_mul else nc.vector
                            meng.tensor_mul(pm[:], pm[:], m_cur[skc][:])
                            pms[skc] = pm
                            # refresh this mask slot with the next block's
                            # half once its last reader is emitted
                            if sqb == 0 and hi == NH - 1 and NSQB > 1:
                                m_next[skc] = mask_dma(skc, 1)
                            # previous head's norm: evac right after the PV
                            # flushes (frees its psum slot), remaining
                            # pieces one per iteration
                            if skc >= 1 and carry_norm:
                                carry_norm.pop(0)()
                        # head end: set up finishers for this head
                        assert not carry_pv and not carry_norm
                        carry_pv = [
                            lambda skc=skc_, f=emit_pv, pms=pms: f(pms[skc], skc)
                            for skc_ in (SKT - 2, SKT - 1)
                        ]
                        carry_norm = make_norm(
                            pv, h, ht, q0, fast=last_sqb and hi == NH - 1
                        )

                        if last_sqb and hi == 1:
                            # cc0 half of the last block's O-proj runs
                            # during the last two heads' attention
                            for dc in range(D // P):
                                for half in range(2):
                                    pending.append(
                                        (
                                            lambda dc=dc, half=half, sqb=sqb: emit_oproj_piece(
                                                sqb, dc, half, [0], od3, sqb * SQB
                                            ),
                                            True,
                                        )
                                    )
                    # end heads
                    if not last_sqb:
                        # whole-block O-proj pieces drained in the next block
                        for dc in range(D // P):
                            for half in range(2):
                                pending.append(
                                    (
                                        lambda sqb=sqb, dc=dc, half=half: emit_oproj_piece(
                                            sqb, dc, half, [0, 1], od3, sqb * SQB
                                        ),
                                        True,
                                    )
                                )
                # final head: flush PVs and run its norm immediately, then
                # the cc1 tail pieces (half-outer so half0 starts after the
                # first mul), written to the odx partial
                for f in carry_pv:
                    f()
                carry_pv = []
                for fn, _ in pending:
                    fn()
                pending = []
                ev0, r0, m0, ev1, r1, m1 = carry_norm
                ev0(); r0(); m0(); ev1(); r1()
                for dc in range(D // P):
                    if dc == 2:
                        m1()
                    emit_oproj_piece(
                        NSQB - 1, dc, 0, [1], odx3, 0,
                        evac="dve" if dc % 2 else "act",
                        pool="pss" if dc % 2 else None,
                        dma=nc.gpsimd if dc % 2 == 0 else nc.sync,
                    )
                for dc in range(D // P):
                    emit_oproj_piece(
                        NSQB - 1, dc, 1, [1], odx3, 0,
                        evac="dve" if dc % 2 else "act",
                        pool="pss" if dc % 2 else None,
                        dma=nc.gpsimd if dc % 2 == 0 else nc.sync,
                    )

    nc.compile()
    return nc


def get_program():
    if "nc" not in _CACHE:
        _CACHE["nc"] = _build_program()
    return _CACHE["nc"]


def make_in_maps(q, k, v, mask, Wq, bq, Wk, bk, Wv, bv, Wo, bo, gate):
    """Host-side sharding: per-core input dict (all numpy)."""
    q, k, v = (np.asarray(a, np.float32) for a in (q, k, v))
    mask = np.asarray(mask)
    Wq, bq, Wk, bk, Wv, bv, Wo, bo, gate = (
        np.asarray(a, np.float32) for a in (Wq, bq, Wk, bk, Wv, bv, Wo, bo, gate)
    )
    scale = 1.0 / np.sqrt(HD)
    xT = {}
    for b in range(B):
        xT[("q", b)] = np.ascontiguousarray(q[b].T).astype(NPBF16)
        xT[("k", b)] = np.ascontiguousarray(k[b].T).astype(NPBF16)
        xT[("v", b)] = np.ascontiguousarray(v[b].T).astype(NPBF16)
        xT[("m", b)] = np.ascontiguousarray(mask[b].T).astype(NPBF16)

    in_maps = []
    for c in range(NCORES):
        b = c // (NCORES // B)
        g = c % (NCORES // B)
        cols = slice(g * COLS, (g + 1) * COLS)
        gate_cols = np.repeat(gate[g * NH : (g + 1) * NH], HD)  # [256]
        in_maps.append(
            {
                "xqT": xT[("q", b)],
                "xkT": xT[("k", b)],
                "xvT": xT[("v", b)],
                "mT": xT[("m", b)],
                # fold the 1/sqrt(hd) score scale into Wq and bq;
                # fold the per-head gate into Wv (bv handled on host)
                "wq": (Wq[:, cols] * scale).astype(NPBF16),
                "wk": Wk[:, cols].astype(NPBF16),
                "wv": (Wv[:, cols] * gate_cols[None, :]).astype(NPBF16),
                "wo": np.ascontiguousarray(Wo[cols, :]).astype(NPBF16),
                "bq": (bq[cols] * scale).astype(np.float32).reshape(COLS, 1),
                "bk": bk[cols].astype(np.float32).reshape(COLS, 1),
            }
        )
    return in_maps


LAST_RESULTS = None


def kernel(q, k, v, mask, Wq, bq, Wk, bk, Wv, bv, Wo, bo, gate, trace=False):
    global LAST_RESULTS
    nc = get_program()
    in_maps = make_in_maps(q, k, v, mask, Wq, bq, Wk, bk, Wv, bv, Wo, bo, gate)
    res = run_bass_kernel_spmd(nc, in_maps, core_ids=list(range(NCORES)), trace=trace)
    LAST_RESULTS = res

    bv_ = np.asarray(bv, np.float32)
    bo_ = np.asarray(bo, np.float32)
    gate_ = np.asarray(gate, np.float32)
    Wo_ = np.asarray(Wo, np.float32)
    # attention rows sum to 1, so the bv term is a constant vector:
    # concat-level constant = repeat(gate, hd) * bv, projected through Wo.
    const = (np.repeat(gate_, HD) * bv_) @ Wo_ + bo_

    out = np.zeros((B, S, D), np.float32)
    for c in range(NCORES):
        b = c // (NCORES // B)
        out[b] += res.results[c]["od"].astype(np.float32).T
        # the last sq block's cc1 contribution is a separate partial
        out[b, (NSQB - 1) * SQB :, :] += res.results[c]["odx"].astype(np.float32).T
    out += const[None, None, :]
    return out


# revision 107
# speedup vs baseline: 1.1521x; 1.0042x over previous
"""Multi-head attention on 8 Trainium2 NeuronCores.

Problem: B=2, S=2048, D=1024, H=16 heads (head_dim 64), boolean mask,
per-head gate, QKV/out linear projections.

Sharding: core c handles batch b=c//4 and heads 4*(c%4)..4*(c%4)+3.
Each core computes its 4 heads' attention and the partial output
projection (contribution of its 256 concat columns through Wo); the host
sums the 4 partials per batch and adds the constant terms (bo, and the
bv/gate contribution which is constant because attention rows sum to 1).

Schedule (PE-bound design, tuned against the CoreSim cost model; all
constructs are neuronxcc/BIR-legal — gpsimd never touches PSUM, no f32r):
  - ACT (scalar) engine runs the 128 x [128,1024] exp stream plus only
    the xk/xq DMAs that complete before the first exp.  Other DMAs live
    on the SP and Pool queues; each weight tensor is one strided DMA.
  - PE is in-order, so emission order IS the schedule.  Only the c0
    halves of the K/Q projections run before attention (first scores at
    ~20us); the c1 halves run as deferred [128,512] pieces inside the
    h0/h1 PE slack (K-c1 interleaves into Q-c0's DMA-gated stalls).
  - scores computed transposed [sk, sq]; exp is a pure ACT pass, mask is
    a multiplicative fp16 tensor_mul on DVE, softmax denominator rides as
    a 65th "ones" column of the PV stationary.  PV trails scores by two
    chunks and the last two PVs + the (5-piece, 512-halved) normalization
    of each head spread across the next head's iterations, so the psum
    slots hand over without stalling PE.
  - V projection is single-pass (skt-outer, xv resident) into 1-bank
    psum scratch slots during h0; evacuation on ACT (slack there).
  - O projection is cut into [128,512] 1-bank pieces drained one per two
    iterations: sqb0's pieces into sqb1's h0/h1, the last block's cc0
    pieces into its h3/h2, and its cc1 pieces form a pipelined tail
    written to the odx partial (summed on host).  Mask tiles are resident
    per-sq-block only (halved footprint funds xv's own sbuf slots).
  - od/odx partials are written fp16 (host accumulates in fp32).
"""

import sys

if "/opt/trn_rl_repo" not in sys.path:
    sys.path.insert(0, "/opt/trn_rl_repo")

import numpy as np

import concourse.bass as bass
import concourse.bacc as bacc
import concourse.mybir as mybir
import concourse.tile as tile
from concourse.bass_utils import run_bass_kernel_spmd

BF16 = mybir.dt.float16  # fp16: same speed as bf16, 3 more mantissa bits
F32 = mybir.dt.float32
NPBF16 = np.float16

P = 128
B, S, D = 2, 2048, 1024
HEADS, HD = 16, 64
NCORES = 8
NH = HEADS // (NCORES // B)  # heads per core = 4
COLS = NH * HD               # 256 concat columns per core
DK = D // P                  # 8 contraction chunks for the projections
SKT = S // P                 # 16 key chunks
SQB = 1024                   # query block width in the attention loop
NSQB = S // SQB

_CACHE = {}


def _build_program():
    nc = bacc.Bacc("TRN2", debug=False)

    xqT = nc.declare_dram_parameter("xqT", [D, S], BF16, isOutput=False)
    xkT = nc.declare_dram_parameter("xkT", [D, S], BF16, isOutput=False)
    xvT = nc.declare_dram_parameter("xvT", [D, S], BF16, isOutput=False)
    mT = nc.declare_dram_parameter("mT", [S, S], BF16, isOutput=False)
    wq = nc.declare_dram_parameter("wq", [D, COLS], BF16, isOutput=False)
    wk = nc.declare_dram_parameter("wk", [D, COLS], BF16, isOutput=False)
    wv = nc.declare_dram_parameter("wv", [D, COLS], BF16, isOutput=False)
    wo = nc.declare_dram_parameter("wo", [COLS, D], BF16, isOutput=False)
    bq = nc.declare_dram_parameter("bq", [COLS, 1], F32, isOutput=False)
    bk = nc.declare_dram_parameter("bk", [COLS, 1], F32, isOutput=False)
    od = nc.declare_dram_parameter("od", [D, S], BF16, isOutput=True)
    # cc1 (heads 2,3) partial of the LAST sq block, summed on host
    odx = nc.declare_dram_parameter("odx", [D, SQB], BF16, isOutput=True)

    xqT3 = xqT[:].rearrange("(n p) s -> n p s", p=P)
    xkT3 = xkT[:].rearrange("(n p) s -> n p s", p=P)
    xvT3 = xvT[:].rearrange("(n p) s -> n p s", p=P)
    mT3 = mT[:].rearrange("(n p) s -> n p s", p=P)
    wq3 = wq[:].rearrange("(n p) c -> n p c", p=P)
    wk3 = wk[:].rearrange("(n p) c -> n p c", p=P)
    wv3 = wv[:].rearrange("(n p) c -> n p c", p=P)
    wo3 = wo[:].rearrange("(n p) d -> n p d", p=P)
    bq3 = bq[:].rearrange("(n p) o -> n p o", p=P)
    bk3 = bk[:].rearrange("(n p) o -> n p o", p=P)
    od3 = od[:].rearrange("(n p) s -> n p s", p=P)
    odx3 = odx[:].rearrange("(n p) s -> n p s", p=P)

    with tile.TileContext(nc) as tc:
        with (
            tc.tile_pool(name="wpool", bufs=1) as wpool,
            tc.tile_pool(name="qkpool", bufs=1) as qkpool,
            tc.tile_pool(name="vpool", bufs=1) as vpool,
            tc.tile_pool(name="maskpool", bufs=1) as maskpool,
            tc.tile_pool(name="cpool", bufs=1) as cpool,
            tc.tile_pool(name="xpool", bufs=1) as xpool,
            tc.tile_pool(name="pmpool", bufs=1) as pmpool,
            tc.tile_pool(name="npool", bufs=1) as npool,
            tc.tile_pool(name="opool", bufs=1) as opool,
        ):
            # ---------------- DMA issue (t=0), one queue per engine -------
            # SP: weights/biases, then xv (reusing xk's sbuf slots), later od.
            # ACT: xk only (completes before the first exp).
            # DVE: xq only (completes before the first mask-mul).
            # Pool: the 16 mask tiles.
            # each weight tensor is ONE strided DMA into a [128, DK*COLS]
            # tile (8 small DMAs would pay ~0.8us of HWDGE overhead each)
            wk3b = wk[:].rearrange("(n p) c -> p n c", p=P)
            wq3b = wq[:].rearrange("(n p) c -> p n c", p=P)
            wv3b = wv[:].rearrange("(n p) c -> p n c", p=P)
            wo3b = wo[:].rearrange("(n p) d -> p n d", p=P)

            # tiny dk0 chunk first so the very first ldweights isn't
            # gated by the full strided wk transfer
            wk0_sb = wpool.tile([P, COLS], BF16, name="wk0_sb")
            nc.sync.dma_start(out=wk0_sb[:], in_=wk3[0])
            wk_all = wpool.tile([P, DK, COLS], BF16, name="wk_all")
            nc.sync.dma_start(out=wk_all[:], in_=wk3b)
            wk_sb = [wk_all[:, i, :] for i in range(DK)]
            wk_sb[0] = wk0_sb[:]
            b_sb = {}
            for nm, src in (("bk", bk3), ("bq", bq3)):
                for i in range(COLS // P):
                    t = wpool.tile([P, 1], F32, name=f"{nm}_sb{i}")
                    nc.sync.dma_start(out=t[:], in_=src[i])
                    b_sb[(nm, i)] = t
            wq_all = wpool.tile([P, DK, COLS], BF16, name="wq_all")
            nc.sync.dma_start(out=wq_all[:], in_=wq3b)
            wq_sb = [wq_all[:, i, :] for i in range(DK)]

            # x streams split across queues so neither gates a projection:
            # ACT takes xk0-3 + xq0-3 (done well before the first exp), SP
            # takes xk4-7 + xq4-7 between the weight loads, Pool takes xv
            # (reusing the xk sbuf slots once K-proj drains them) and then
            # the 16 mask tiles.
            xk_t, xq_t, xv_t = [], [], []
            for i in range(DK):
                t = xpool.tile([P, S], BF16, name="xk", tag=f"xk{i}", bufs=1)
                if i == 0:
                    # xk0 gates the very first matmuls: two half DMAs on
                    # the Pool/SWDGE queue land the first 1024 columns
                    # ~1.5us sooner than one ACT HWDGE transfer
                    nc.gpsimd.dma_start(out=t[:, 0:1024], in_=xkT3[0][:, 0:1024])
                    nc.gpsimd.dma_start(out=t[:, 1024:2048], in_=xkT3[0][:, 1024:2048])
                else:
                    (nc.scalar if i < 4 else nc.sync).dma_start(out=t[:], in_=xkT3[i])
                xk_t.append(t)
            for i in range(DK):
                t = xpool.tile([P, S], BF16, name="xq", tag=f"xq{i}", bufs=1)
                (nc.scalar if i < 4 else nc.sync).dma_start(out=t[:], in_=xqT3[i])
                xq_t.append(t)
            for i in range(DK):
                t = xpool.tile([P, S], BF16, name="xv", tag=f"xv{i}", bufs=1)
                nc.gpsimd.dma_start(out=t[:], in_=xvT3[i])
                xv_t.append(t)

            # mask tiles are resident only for the CURRENT sq block
            # ([128, SQB] halves, one generation per block) — this halves
            # their sbuf footprint so xv gets its own slots above.  The
            # sqb1 generation is DMA'd during sqb0's last head.
            def mask_dma(skc, sqb):
                t = maskpool.tile(
                    [P, SQB], BF16, name=f"m{skc}", tag=f"m{skc}", bufs=1
                )
                nc.gpsimd.dma_start(
                    out=t[:], in_=mT3[skc][:, sqb * SQB : (sqb + 1) * SQB]
                )
                return t

            m_sb = [mask_dma(i, 0) for i in range(SKT)]

            # wv/wo follow the x streams on SP (needed at ~31us / ~85us)
            wv_all = wpool.tile([P, DK, COLS], BF16, name="wv_all")
            nc.sync.dma_start(out=wv_all[:], in_=wv3b)
            wv_sb = [wv_all[:, i, :] for i in range(DK)]
            wo_all = wpool.tile([P, COLS // P, D], BF16, name="wo_all")
            nc.sync.dma_start(out=wo_all[:], in_=wo3b)
            wo_sb = [wo_all[:, i, :] for i in range(COLS // P)]

            # Preload the ACT exp table (~1.3us) while ACT is otherwise idle
            # (right after its xk DMAs), so the first exp doesn't pay it.
            warm = npool.tile([P, 1], F32, name="warm", tag="warm", bufs=1)
            nc.scalar.activation(
                warm[:], b_sb[("bk", 0)][:], mybir.ActivationFunctionType.Exp
            )

            # concat^T (normalized attention outputs, head-major columns)
            concat_sb = [
                cpool.tile([P, S], BF16, name=f"concat_sb{i}")
                for i in range(COLS // P)
            ]

            # ---------------- K then Q projections ----------------------
            # qhT[c, s] = (x @ W + b)^T: lhsT = W chunk [128d, 128c]
            # (stationary), rhs = xT chunk [128d, 512s] -> psum [128c, 512s],
            # accumulated over the 8 d-chunks.  K first (its DMA stream and
            # weights land first), then Q reusing the same psum banks.
            # Only the c0 halves of the K/Q projections (heads 0/1) run
            # before attention; the c1 halves are deferred [128,512]-chunk
            # pieces drained into h0/h1's PE slack (they are needed first
            # by h2, ~35us later).  PE is in-order, so this pulls the first
            # scores ~12us earlier.  Evac+bias runs in [128,512] quarters
            # so the psum bank's next user starts a quarter-latency later.
            qhT_sb = {}
            qhT_sb[("k", 1)] = qkpool.tile([P, S], BF16, name="khT1")
            qhT_sb[("q", 1)] = qkpool.tile([P, S], BF16, name="qhT1")
            with tc.tile_pool(name="ps_proj", bufs=1, space="PSUM") as psp:
                def early_c1_piece(tname, x_t, w_sb, sb):
                    pst = psp.tile([P, 512], F32, name="pc1e", tag="pc1e", bufs=2)
                    for dk in range(DK):
                        nc.tensor.matmul(
                            pst[:],
                            lhsT=w_sb[dk][:, P:COLS],
                            rhs=x_t[dk][:, sb * 512 : (sb + 1) * 512],
                            start=(dk == 0),
                            stop=(dk == DK - 1),
                        )
                    # evac on ACT (idle between its DMA stream and the
                    # first exp), keeping DVE clear for Q-c0's bias which
                    # gates the first scores
                    nc.scalar.add(
                        qhT_sb[(tname, 1)][:, sb * 512 : (sb + 1) * 512],
                        pst[:],
                        b_sb[(f"b{tname}", 1)][:],
                    )

                for tname, x_t, w_sb in (("k", xk_t, wk_sb), ("q", xq_t, wq_sb)):
                    pst = psp.tile([P, S], F32, name=f"psp_{tname}0", tag="psp0")
                    for dk in range(DK):
                        for sb in range(S // 512):
                            nc.tensor.matmul(
                                pst[:, sb * 512 : (sb + 1) * 512],
                                lhsT=w_sb[dk][:, 0:P],
                                rhs=x_t[dk][:, sb * 512 : (sb + 1) * 512],
                                start=(dk == 0),
                                stop=(dk == DK - 1),
                            )
                        # Q-c0's dk matmuls are gated by the xq DMA stream;
                        # fill the PE stalls with K-c1 chunk pieces (their
                        # xk inputs are already resident)
                        if tname == "q" and 1 <= dk <= 4:
                            early_c1_piece("k", xk_t, wk_sb, dk - 1)
                    t = qkpool.tile([P, S], BF16, name=f"{tname}hT0")
                    for qtr in range(4):
                        hs = slice(qtr * 512, (qtr + 1) * 512)
                        nc.vector.tensor_scalar_add(
                            t[:, hs], pst[:, hs], b_sb[(f"b{tname}", 0)][:]
                        )
                    qhT_sb[(tname, 0)] = t

            # ---------------- attention ---------------------------------
            # sqb-outer / head-inner.  V-projection pieces are interleaved
            # into (sqb0, h0); O-projection of sqb_i is interleaved into
            # sqb_{i+1}'s attention; the last sqb's O-proj is split by
            # head-pair with the cc1 half as the (pipelined) tail.
            vh_sb = [None] * SKT

            with (
                tc.tile_pool(name="ps_s", bufs=1, space="PSUM") as ps_s_pool,
                tc.tile_pool(name="ps_pv", bufs=1, space="PSUM") as ps_pv_pool,
            ):
                def emit_vproj(skt):
                    # one V-proj piece: vh[skt] = (xv @ Wv)[skt block] + ones
                    psv = ps_pv_pool.tile(
                        [P, 512], F32, name="psv", tag="scratch", bufs=2
                    )
                    for dk in range(DK):
                        nc.tensor.matmul(
                            psv[:, 0:COLS],
                            lhsT=xv_t[dk][:, skt * P : (skt + 1) * P],
                            rhs=wv_sb[dk][:],
                            start=(dk == 0),
                            stop=(dk == DK - 1),
                        )
                    vt = vpool.tile([P, NH, HD + 1], BF16, name=f"vh_sb{skt}")
                    nc.scalar.copy(
                        vt[:, :, 0:HD],
                        psv[:, 0:COLS].rearrange("p (h d) -> p h d", h=NH),
                    )
                    nc.gpsimd.memset(vt[:, :, HD], 1.0)
                    vh_sb[skt] = vt

                def emit_oproj_piece(sqb, dc, half, ccs, dest3, dq0, evac="pool", dma=None, pool=None):
                    # po[128d, 512sq] = sum_cc wo_cc^T @ concat_cc
                    q0 = sqb * SQB + half * 512
                    if pool == "pss":
                        # tail only: the attention's score psum banks are
                        # dead, reuse them as extra O-proj slots
                        po_w = ps_s_pool.tile([P, SQB], F32, name="pss", tag="pss", bufs=2)
                        po = po_w[:, 0:512]
                    else:
                        po = ps_pv_pool.tile(
                            [P, 512], F32, name="po", tag="scratch", bufs=2
                        )[:]
                    for j, cc in enumerate(ccs):
                        nc.tensor.matmul(
                            po[:],
                            lhsT=wo_sb[cc][:, dc * P : (dc + 1) * P],
                            rhs=concat_sb[cc][:, q0 : q0 + 512],
                            start=(j == 0),
                            stop=(j == len(ccs) - 1),
                        )
                    oev = opool.tile([P, 512], BF16, name="oev", tag="oev", bufs=6)
                    if evac == "act":
                        nc.scalar.copy(oev[:], po)
                    else:
                        nc.vector.tensor_copy(oev[:], po)
                    (dma or nc.sync).dma_start(
                        out=dest3[dc][:, dq0 + half * 512 : dq0 + half * 512 + 512],
                        in_=oev[:],
                    )

                ones64 = npool.tile([HD + 1, HD], F32, name="ones64", tag="ones64", bufs=1)
                nc.vector.memset(ones64[:], 1.0)

                # deferred PE pieces, drained one per skc iteration.  An
                # entry flagged needs_concat (O-proj) may only run once the
                # previous head's concat writes are all emitted.
                pending = []

                def drain(n):
                    k = 0
                    while pending and k < n:
                        fn, needs_concat = pending[0]
                        if needs_concat and carry_norm:
                            break
                        pending.pop(0)
                        fn()
                        k += 1

                def emit_proj_piece(tname, x_t, w_sb, sb):
                    # c1 half of the K/Q projection, one [128,512] chunk
                    pst = ps_pv_pool.tile(
                        [P, 512], F32, name="pc1", tag="scratch", bufs=2
                    )
                    for dk in range(DK):
                        nc.tensor.matmul(
                            pst[:],
                            lhsT=w_sb[dk][:, P:COLS],
                            rhs=x_t[dk][:, sb * 512 : (sb + 1) * 512],
                            start=(dk == 0),
                            stop=(dk == DK - 1),
                        )
                    # evac on ACT (idle between its DMA stream and the
                    # first exp), keeping DVE clear for Q-c0's bias which
                    # gates the first scores
                    nc.scalar.add(
                        qhT_sb[(tname, 1)][:, sb * 512 : (sb + 1) * 512],
                        pst[:],
                        b_sb[(f"b{tname}", 1)][:],
                    )

                # K-c1 ran early (interleaved into Q-c0); Q-c1 chunks are
                # drained into h0/h1's slack (h2 needs chunks 0/1 first)
                for sb in range(4):
                    pending.append(
                        (
                            lambda sb=sb: emit_proj_piece("q", xq_t, wq_sb, sb),
                            False,
                        )
                    )

                def make_norm(pv, h, ht, q0, fast=False):
                    # normalization for head h's pv accumulator, cut into 5
                    # pieces spread over the next head's early iterations:
                    # [dnc evac both halves] [recip0+bcast0] [mul0]
                    # [recip1+bcast1] [mul1].  Evac halves go to Pool+DVE in
                    # parallel so the psum slot frees fast.  The denominator
                    # reciprocal runs in-lane on partition 64 and is
                    # broadcast to partitions 0-63 by a K=1 f32r matmul into
                    # a scratch psum bank.  Odd heads land in concat rows
                    # 64:128 via a gpsimd DMA hop (DVE is lane-locked).
                    dnc = npool.tile(
                        [HD + 1, SQB], F32, name="dnc", tag="dnc", bufs=2
                    )

                    def evac_half(i):
                        def fn():
                            nc.vector.tensor_copy(
                                dnc[:, i * 512 : (i + 1) * 512], pv[i][:]
                            )
                        return fn

                    def evac():
                        evac_half(0)()
                        evac_half(1)()

                    rbs = {}

                    def recip_piece(half):
                        def fn():
                            sl = slice(half * 512, half * 512 + 512)
                            if fast:
                                # tail-critical: in-lane reciprocal on
                                # partition 64 + fp32 K=1 broadcast matmul
                                # (shorter chain than the hop+broadcast)
                                nc.vector.reciprocal(
                                    out=dnc[HD : HD + 1, sl], in_=dnc[HD : HD + 1, sl]
                                )
                                rb = ps_pv_pool.tile(
                                    [P, 512], F32, name="rbf", tag="scratch", bufs=2
                                )
                                nc.tensor.matmul(
                                    rb[0:HD, :],
                                    lhsT=ones64[HD : HD + 1, :],
                                    rhs=dnc[HD : HD + 1, sl],
                                    start=True,
                                    stop=True,
                                )
                                rbs[half] = rb[0:HD, :]
                                return
                            # hop the denominator row to partition 0 (DVE is
                            # lane-locked; reciprocal_approx_fast and
                            # partition_broadcast are partition-0 ops)
                            dn0 = npool.tile([1, 512], F32, name="dn0", tag="dn0", bufs=1)
                            nc.gpsimd.dma_start(out=dn0[:], in_=dnc[HD : HD + 1, sl])
                            nc.vector.reciprocal_approx_fast(out=dn0[:], in_=dn0[:])
                            rb = npool.tile(
                                [HD, 512], F32, name=f"rb{half}", tag=f"rb{half}", bufs=1
                            )
                            nc.gpsimd.partition_broadcast(rb[:], dn0[:])
                            rbs[half] = rb[:]
                        return fn

                    def mul_piece(half):
                        def fn():
                            sl = slice(half * 512, half * 512 + 512)
                            cs = slice(q0 + half * 512, q0 + half * 512 + 512)
                            if h % 2 == 0:
                                nc.vector.tensor_mul(
                                    concat_sb[ht][0:HD, cs], dnc[0:HD, sl], rbs[half]
                                )
                            else:
                                tmp = npool.tile([HD, 512], BF16, name="tmpn", tag="tmpn", bufs=1)
                                nc.vector.tensor_mul(tmp[:], dnc[0:HD, sl], rbs[half])
                                nc.gpsimd.dma_start(out=concat_sb[ht][HD:P, cs], in_=tmp[:])
                        return fn

                    if fast:
                        # tail: per-half chains so the first O-proj pieces
                        # start after half0's mul instead of the full chain
                        return [
                            evac_half(0), recip_piece(0), mul_piece(0),
                            evac_half(1), recip_piece(1), mul_piece(1),
                        ]
                    return [evac, recip_piece(0), mul_piece(0), recip_piece(1), mul_piece(1)]

                # carried finishers from the previous head:
                # [PV(prev,14), PV(prev,15)] then the 5 norm pieces
                carry_pv = []
                carry_norm = []

                m_cur = m_sb
                m_next = [None] * SKT
                for sqb in range(NSQB):
                    q0 = sqb * SQB
                    last_sqb = sqb == NSQB - 1
                    if sqb == 1:
                        m_cur = m_next
                    heads = [0, 1, 3, 2] if last_sqb else [0, 1, 2, 3]
                    for hi, h in enumerate(heads):
                        ht, hp = h // 2, HD * (h % 2)
                        qT = qhT_sb[("q", ht)]
                        kT = qhT_sb[("k", ht)]
                        pv = [
                            ps_pv_pool.tile(
                                [HD + 1, 512], F32, name=f"pspv{i}", tag=f"pspv{i}", bufs=1
                            )
                            for i in range(2)
                        ]

                        def emit_pv(pm_t, skc, pv=pv, h=h):
                            for i in range(SQB // 512):
                                nc.tensor.matmul(
                                    pv[i][:],
                                    lhsT=vh_sb[skc][:, h, :],
                                    rhs=pm_t[:, i * 512 : (i + 1) * 512],
                                    start=(skc == 0),
                                    stop=(skc == SKT - 1),
                                )

                        pms = [None] * SKT
                        for skc in range(SKT):
                            ss = ps_s_pool.tile(
                                [P, SQB], F32, name="pss", tag="pss", bufs=2
                            )
                            for i in range(SQB // 512):
                                nc.tensor.matmul(
                                    ss[:, i * 512 : (i + 1) * 512],
                                    lhsT=kT[hp : hp + HD, skc * P : (skc + 1) * P],
                                    rhs=qT[hp : hp + HD, q0 + i * 512 : q0 + (i + 1) * 512],
                                    start=True,
                                    stop=True,
                                )
                            # PE-order fillers: carried PV flushes of the
                            # previous head at skc 0/1, then this head's PV
                            # trailing by two chunks; V-proj pieces ride in
                            # (sqb0, h0); O-proj pieces drain once concat of
                            # their block is complete (skc>=6 guard).
                            if skc <= 1 and carry_pv:
                                carry_pv.pop(0)()
                            if sqb == 0 and hi == 0:
                                emit_vproj(skc)
                            elif skc % 2 == 1 or (last_sqb and hi >= 2 and len(pending) >= 1):
                                drain(1)
                            if skc >= 2:
                                emit_pv(pms[skc - 2], skc - 2)
                            pm = pmpool.tile([P, SQB], BF16, name="pm", tag="pm", bufs=4)
                            if last_sqb and hi == NH - 1 and skc == SKT - 1:
                                # final chunk: exp in halves so the last PV
                                # (and the tail behind it) starts earlier
                                for eh in range(2):
                                    es = slice(eh * 512, eh * 512 + 512)
                                    nc.scalar.activation(
                                        pm[:, es], ss[:, es],
                                        mybir.ActivationFunctionType.Exp,
                                    )
                            else:
                                nc.scalar.activation(
                                    pm[:], ss[:], mybir.ActivationFunctionType.Exp
                                )
                            # every 4th mask-mul rides on the idle gpsimd
                            # engine to relieve DVE pacing (pm is consumed
                            # two iterations later, so the slower engine's
                            # latency is hidden)
                            # higher gpsimd share where DVE also carries
                            # the previous block's O-proj evacuations
                            pool_mul = skc % 4 == 3 or (
                                sqb == 1 and hi <= 1 and skc % 4 == 1
                            )
                            meng = nc.gpsimd if pool_mul else nc.vector
                            meng.tensor_mul(pm[:], pm[:], m_cur[skc][:])
                            pms[skc] = pm
                            # refresh this mask slot with the next block's
                            # half once its last reader is emitted
                            if sqb == 0 and hi == NH - 1 and NSQB > 1:
                                m_next[skc] = mask_dma(skc, 1)
                            # previous head's norm: evac right after the PV
                            # flushes (frees its psum slot), remaining
                            # pieces one per iteration
                            if skc >= 1 and carry_norm:
                                carry_norm.pop(0)()
                        # head end: set up finishers for this head
                        assert not carry_pv and not carry_norm
                        carry_pv = [
                            lambda skc=skc_, f=emit_pv, pms=pms: f(pms[skc], skc)
                            for skc_ in (SKT - 2, SKT - 1)
                        ]
                        carry_norm = make_norm(
                            pv, h, ht, q0, fast=last_sqb and hi == NH - 1
                        )

                        if last_sqb and hi == 1:
                            # cc0 half of the last block's O-proj runs
                            # during the last two heads' attention
                            for dc in range(D // P):
                                for half in range(2):
                                    pending.append(
                                        (
                                            lambda dc=dc, half=half, sqb=sqb: emit_oproj_piece(
                                                sqb, dc, half, [0], od3, sqb * SQB
                                            ),
                                            True,
                                        )
                                    )
                    # end heads
                    if not last_sqb:
                        # whole-block O-proj pieces drained in the next block
                        for dc in range(D // P):
                            for half in range(2):
                                pending.append(
                                    (
                                        lambda sqb=sqb, dc=dc, half=half: emit_oproj_piece(
                                            sqb, dc, half, [0, 1], od3, sqb * SQB
                                        ),
                                        True,
                                    )
                                )
                # final head: flush PVs and run its norm immediately, then
                # the cc1 tail pieces (half-outer so half0 starts after the
                # first mul), written to the odx partial
                for f in carry_pv:
                    f()
                carry_pv = []
                for fn, _ in pending:
                    fn()
                pending = []
                ev0, r0, m0, ev1, r1, m1 = carry_norm
                ev0(); r0(); m0(); ev1(); r1()
                for dc in range(D // P):
                    if dc == 2:
                        m1()
                    emit_oproj_piece(
                        NSQB - 1, dc, 0, [1], odx3, 0,
                        evac="dve" if dc % 2 else "act",
                        pool="pss" if dc % 2 else None,
                        dma=nc.gpsimd if dc in (0, 1, 2, 4, 5) else nc.sync,
                    )
                for dc in range(D // P):
                    emit_oproj_piece(
                        NSQB - 1, dc, 1, [1], odx3, 0,
                        evac="dve" if dc % 2 else "act",
                        pool="pss" if dc % 2 else None,
                        dma=nc.gpsimd if dc in (0, 1, 2, 4, 5) else nc.sync,
                    )

    nc.compile()
    return nc


def get_program():
    if "nc" not in _CACHE:
        _CACHE["nc"] = _build_program()
    return _CACHE["nc"]


def make_in_maps(q, k, v, mask, Wq, bq, Wk, bk, Wv, bv, Wo, bo, gate):
    """Host-side sharding: per-core input dict (all numpy)."""
    q, k, v = (np.asarray(a, np.float32) for a in (q, k, v))
    mask = np.asarray(mask)
    Wq, bq, Wk, bk, Wv, bv, Wo, bo, gate = (
        np.asarray(a, np.float32) for a in (Wq, bq, Wk, bk, Wv, bv, Wo, bo, gate)
    )
    scale = 1.0 / np.sqrt(HD)
    xT = {}
    for b in range(B):
        xT[("q", b)] = np.ascontiguousarray(q[b].T).astype(NPBF16)
        xT[("k", b)] = np.ascontiguousarray(k[b].T).astype(NPBF16)
        xT[("v", b)] = np.ascontiguousarray(v[b].T).astype(NPBF16)
        xT[("m", b)] = np.ascontiguousarray(mask[b].T).astype(NPBF16)

    in_maps = []
    for c in range(NCORES):
        b = c // (NCORES // B)
        g = c % (NCORES // B)
        cols = slice(g * COLS, (g + 1) * COLS)
        gate_cols = np.repeat(gate[g * NH : (g + 1) * NH], HD)  # [256]
        in_maps.append(
            {
                "xqT": xT[("q", b)],
                "xkT": xT[("k", b)],
                "xvT": xT[("v", b)],
                "mT": xT[("m", b)],
                # fold the 1/sqrt(hd) score scale into Wq and bq;
                # fold the per-head gate into Wv (bv handled on host)
                "wq": (Wq[:, cols] * scale).astype(NPBF16),
                "wk": Wk[:, cols].astype(NPBF16),
                "wv": (Wv[:, cols] * gate_cols[None, :]).astype(NPBF16),
                "wo": np.ascontiguousarray(Wo[cols, :]).astype(NPBF16),
                "bq": (bq[cols] * scale).astype(np.float32).reshape(COLS, 1),
                "bk": bk[cols].astype(np.float32).reshape(COLS, 1),
            }
        )
    return in_maps


LAST_RESULTS = None


def kernel(q, k, v, mask, Wq, bq, Wk, bk, Wv, bv, Wo, bo, gate, trace=False):
    global LAST_RESULTS
    nc = get_program()
    in_maps = make_in_maps(q, k, v, mask, Wq, bq, Wk, bk, Wv, bv, Wo, bo, gate)
    res = run_bass_kernel_spmd(nc, in_maps, core_ids=list(range(NCORES)), trace=trace)
    LAST_RESULTS = res

    bv_ = np.asarray(bv, np.float32)
    bo_ = np.asarray(bo, np.float32)
    gate_ = np.asarray(gate, np.float32)
    Wo_ = np.asarray(Wo, np.float32)
    # attention rows sum to 1, so the bv term is a constant vector:
    # concat-level constant = repeat(gate, hd) * bv, projected through Wo.
    const = (np.repeat(gate_, HD) * bv_) @ Wo_ + bo_

    out = np.zeros((B, S, D), np.float32)
    for c in range(NCORES):
        b = c // (NCORES // B)
        out[b] += res.results[c]["od"].astype(np.float32).T
        # the last sq block's cc1 contribution is a separate partial
        out[b, (NSQB - 1) * SQB :, :] += res.results[c]["odx"].astype(np.float32).T
    out += const[None, None, :]
    return out
